# revision 1
# baseline (speedup 1.0000x reference)
"""Causal self-attention (B=2, T=2048, E=1024, H=16, D=64) on 8 TRN2 NeuronCores.

Sharding: core = (batch b, head-group hg): 2 batches x 4 head-groups of 4 heads.
Each core computes QKV projections for its 4 heads (256 columns), causal
attention, and the output projection against its 256 rows of Wo, producing a
partial [2048, 1024] output. Host sums the 4 head-group partials per batch
(the tensor-parallel all-reduce) and adds bo.

Per-core kernel (fp16 matmul operands, fp32 PSUM accumulation):
  - Q^T / K^T computed directly transposed ([256, 2048]) so attention scores
    S^T = K @ Q^T need no transposes anywhere.
  - V in natural layout with a ones column appended: the attn @ V matmul
    also yields softmax row-sums for free.
  - exp on ScalarE (scale=1/8 folded in), no max-subtraction (scores provably
    small), causal block-skipping, diagonal tiles masked multiplicatively
    (with exp trimmed to the valid column range on the top diagonal tiles).
  - Normalization: reciprocal of the rowsums on a [64,16] repartition (DVE
    reciprocal costs by free-size), broadcast via a DRAM bounce, applied on
    GpSimd; the output projection consumes attn^T directly.
  - Emission order software-pipelines scores ahead of attn@V and hides the
    V / pair-1 QKV / output projections under the exp-bound attention phase.
"""
from contextlib import ExitStack

import numpy as np

import concourse.bass as bass  # noqa: F401
import concourse.mybir as mybir
import concourse.tile as tile
from concourse import bacc
from concourse.bass_utils import run_bass_kernel_spmd

T = 2048
E = 1024
HPC = 4          # heads per core
D = 64
S = HPC * D      # 256: per-core head-column slice
KE = E // 128    # 8 contraction tiles for the projections
NKT = T // 128   # 16 key row tiles
NQB = T // 512   # 4 query column blocks
F16 = mybir.dt.float16
F32 = mybir.dt.float32
EXP = mybir.ActivationFunctionType.Exp


def build_nc(phases=None):
    nc = bacc.Bacc("TRN2", target_bir_lowering=False, debug=False)
    xT = nc.dram_tensor("xT", [E, T], F16, kind="ExternalInput").ap()
    wq = nc.dram_tensor("wq", [E, S], F16, kind="ExternalInput").ap()
    wk = nc.dram_tensor("wk", [E, S], F16, kind="ExternalInput").ap()
    wv = nc.dram_tensor("wv", [E, S], F16, kind="ExternalInput").ap()
    wo = nc.dram_tensor("wo", [S, E], F16, kind="ExternalInput").ap()
    bq = nc.dram_tensor("bq", [S, 1], F32, kind="ExternalInput").ap()
    bk = nc.dram_tensor("bk", [S, 1], F32, kind="ExternalInput").ap()
    bv = nc.dram_tensor("bv", [1, S], F32, kind="ExternalInput").ap()
    masks = nc.dram_tensor("masks", [4, 128, 1024], F16, kind="ExternalInput").ap()
    out = nc.dram_tensor("out", [T, E], F16, kind="ExternalOutput").ap()

    with tile.TileContext(nc) as tc:
        _emit(nc, tc, xT, wq, wk, wv, wo, bq, bk, bv, masks, out, phases=phases)
    nc.compile()
    return nc


def _emit(nc, tc, xT, wq, wk, wv, wo, bq, bk, bv, masks, out, dbg=None, phases=None):
    ctx = ExitStack()
    consts = ctx.enter_context(tc.tile_pool(name="consts", bufs=1))
    mm_ps = ctx.enter_context(tc.tile_pool(name="mm_ps", bufs=2, space="PSUM"))
    st_ps = ctx.enter_context(tc.tile_pool(name="st_ps", bufs=2, space="PSUM"))
    ot_ps = ctx.enter_context(tc.tile_pool(name="ot_ps", bufs=2, space="PSUM"))
    pt_pool = ctx.enter_context(tc.tile_pool(name="pt", bufs=6))
    sm_pool = ctx.enter_context(tc.tile_pool(name="sm", bufs=8))
    ob_pool = ctx.enter_context(tc.tile_pool(name="ob", bufs=4))
    dr_pool = ctx.enter_context(tc.tile_pool(name="dr", bufs=8, space="DRAM"))

    # --- constant loads ---
    x_sb = consts.tile([128, KE, T], F16)
    wq_sb = consts.tile([128, KE, S], F16)
    wk_sb = consts.tile([128, KE, S], F16)
    wv_sb = consts.tile([128, KE, S], F16)
    wo_sb = consts.tile([128, S // 128, E], F16)
    bq_sb = consts.tile([128, 2], F32)
    bk_sb = consts.tile([128, 2], F32)
    bv_bc = consts.tile([128, S], F32)
    mask_sb = consts.tile([128, 4, 1024], F16)
    qt_sb = consts.tile([128, 2, T], F16)
    kt_sb = consts.tile([128, 2, T], F16)
    v_sb = consts.tile([128, NKT, HPC, D + 1], F16)
    attnT_sb = consts.tile([128, 2, T], F16)

    # batched constant loads, ordered so the first QK group can start after
    # wq + the first x query-block (~1.5 MB) instead of the full x tensor
    xr = xT.rearrange("(k p) n -> p k n", p=128)
    wqr = wq.rearrange("(k p) m -> p k m", p=128)
    # pair-0 halves of wq and x block 0 first: the first QK group needs only
    # wq[:, :, 0:128] and x[:, :, 0:512]
    nc.sync.dma_start(out=wq_sb[:, 0:4, 0:128], in_=wqr[:, 0:4, 0:128])
    nc.sync.dma_start(out=x_sb[:, 0:4, 0:512], in_=xr[:, 0:4, 0:512])
    nc.sync.dma_start(out=wq_sb[:, 4:8, 0:128], in_=wqr[:, 4:8, 0:128])
    nc.sync.dma_start(out=x_sb[:, 4:8, 0:512], in_=xr[:, 4:8, 0:512])
    nc.sync.dma_start(out=wq_sb[:, :, 128:256], in_=wqr[:, :, 128:256])
    nc.sync.dma_start(out=wk_sb, in_=wk.rearrange("(k p) m -> p k m", p=128))
    nc.sync.dma_start(out=wv_sb, in_=wv.rearrange("(k p) m -> p k m", p=128))
    nc.sync.dma_start(out=bq_sb, in_=bq.rearrange("(a p) one -> p (a one)", p=128))
    nc.sync.dma_start(out=bk_sb, in_=bk.rearrange("(a p) one -> p (a one)", p=128))
    nc.sync.dma_start(out=bv_bc, in_=bv.to_broadcast((128, S)))
    nc.sync.dma_start(out=mask_sb, in_=masks.rearrange("r p n -> p r n"))
    for qb in range(1, NQB):
        qs = slice(qb * 512, (qb + 1) * 512)
        nc.sync.dma_start(out=x_sb[:, :, qs], in_=xr[:, :, qs])
    nc.sync.dma_start(out=wo_sb, in_=wo.rearrange("(a p) n -> p a n", p=128))
    nc.vector.memset(v_sb[:, :, :, D : D + 1], 1.0)

    # --- V = x @ wv + bv (natural layout, with ones column) ---
    def emit_v(rts=range(NKT)):
        for rt in rts:
            ps = mm_ps.tile([128, 512], F32, tag="mm", name=f"vps{rt}")
            for ke in range(KE):
                nc.tensor.matmul(
                    ps[:, 0:S],
                    lhsT=x_sb[:, ke, rt * 128 : (rt + 1) * 128],
                    rhs=wv_sb[:, ke, :],
                    start=(ke == 0),
                    stop=(ke == KE - 1),
                )
            nc.vector.tensor_add(
                v_sb[:, rt, :, 0:D],
                ps[:, 0:S].rearrange("p (h d) -> p h d", h=HPC),
                bv_bc.rearrange("p (h d) -> p h d", h=HPC),
            )

    # --- QT/KT = (x @ w + b).T for one pair of heads (128 cols) ---
    def emit_qk_part(p, qb):
        qs = slice(qb * 512, (qb + 1) * 512)
        for w_sb, b_sb, dst, nm in (
            (wq_sb, bq_sb, qt_sb, "q"),
            (wk_sb, bk_sb, kt_sb, "k"),
        ):
            ps = mm_ps.tile([128, 512], F32, tag="mm", name=f"{nm}ps{p}_{qb}")
            for ke in range(KE):
                nc.tensor.matmul(
                    ps,
                    lhsT=w_sb[:, ke, p * 128 : (p + 1) * 128],
                    rhs=x_sb[:, ke, qs],
                    start=(ke == 0),
                    stop=(ke == KE - 1),
                )
            nc.vector.tensor_scalar_add(dst[:, p, qs], ps, b_sb[:, p : p + 1])

    def emit_qk(p):
        for qb in range(NQB):
            emit_qk_part(p, qb)

    # --- attention for pair p (heads 2p, 2p+1), query block qb ---
    def emit_attn(p, qb):
        qs = slice(qb * 512, (qb + 1) * 512)
        nkt = 4 * (qb + 1)
        ots = [
            ot_ps.tile([D + 1, 512], F32, tag="ot", name=f"ot{p}_{qb}_{i}")
            for i in range(2)
        ]

        # process k-tiles with full-width tiles first and last so the
        # accumulation-group start/stop matmuls cover every PSUM column;
        # the middle diagonal tiles are trimmed to their valid columns
        kt_order = [0, 2, 3, 1] if qb == 0 else list(range(1, nkt)) + [0]

        def trim(kt):
            # valid columns of a diagonal tile r start at 128*r
            r = kt - 4 * qb if kt >= 4 * qb else None
            if r is not None and r >= 1 and kt not in (kt_order[0], kt_order[-1]):
                return r, 128 * r
            return r, 0

        def do_st(kt):
            _, off = trim(kt)
            st = st_ps.tile([128, 1024], F32, tag="st", name=f"st{p}_{qb}_{kt}")
            for hh in range(2):
                hs = slice(hh * 64, (hh + 1) * 64)
                nc.tensor.matmul(
                    st[:, hh * 512 + off : (hh + 1) * 512],
                    lhsT=kt_sb[hs, p, kt * 128 : (kt + 1) * 128],
                    rhs=qt_sb[hs, p, qb * 512 + off : (qb + 1) * 512],
                    start=True,
                    stop=True,
                )
            return st

        sts = {kt_order[0]: do_st(kt_order[0])}
        for i, kt in enumerate(kt_order):
            if i + 1 < len(kt_order):
                sts[kt_order[i + 1]] = do_st(kt_order[i + 1])
            st = sts.pop(kt)
            pt = pt_pool.tile([128, 1024], F16, tag="pt", name=f"pt{p}_{qb}_{kt}")
            r, off = trim(kt)
            if off:
                # upper diagonal tiles: exp/mask only the valid slice of both
                # heads in one strided op each; zero-fill is not needed since
                # the OT matmul below is restricted to the same columns
                stv = st.rearrange("p (a n) -> p a n", a=2)
                ptv = pt.rearrange("p (a n) -> p a n", a=2)
                mkv = mask_sb[:, r, :].rearrange("p (a n) -> p a n", a=2)
                nc.scalar.activation(
                    ptv[:, :, off:512], stv[:, :, off:512], EXP, scale=0.125
                )
                nc.vector.tensor_mul(
                    ptv[:, :, off:512], ptv[:, :, off:512], mkv[:, :, off:512]
                )
            else:
                nc.scalar.activation(pt, st, EXP, scale=0.125)
                if r is not None:
                    nc.vector.tensor_mul(pt, pt, mask_sb[:, r, :])
            for hh in range(2):
                nc.tensor.matmul(
                    ots[hh][:, off:512] if off else ots[hh],
                    lhsT=v_sb[:, kt, 2 * p + hh, :],
                    rhs=pt[:, hh * 512 + off : (hh + 1) * 512],
                    start=(kt == kt_order[0]),
                    stop=(kt == kt_order[-1]),
                    skip_group_check=bool(off),
                )
        # normalization: copy both heads' OT out of PSUM, reciprocal of the
        # two rowsum rows in one op, one DRAM-bounce broadcast, two muls
        oc = sm_pool.tile([D + 1, 2, 512], F32, tag="oc", name=f"oc{p}_{qb}")
        for hh in range(2):
            nc.vector.tensor_copy(oc[:, hh, :], ots[hh])
        # reciprocal of the 1024 rowsums: DVE reciprocal is ~6 cycles/elem and
        # costs by free-size, so first spread the row across 64 partitions
        # (SBUF->SBUF DMA repartition), recip [64, 16], then DRAM-bounce to
        # broadcast (DMA cannot broadcast from an SBUF partition, but can
        # from DRAM)
        rsq = sm_pool.tile([D, 16], F32, tag="rsq", name=f"rsq{p}_{qb}")
        nc.sync.dma_start(out=rsq, in_=oc[D : D + 1, :, :])
        rr = sm_pool.tile([D, 16], F16, tag="rr", name=f"rr{p}_{qb}")
        with nc.allow_low_precision(reason="softmax denom reciprocal in fp16"):
            nc.vector.reciprocal(rr, rsq)
        rd = dr_pool.tile([1, 2, 512], F16, tag="rd", name=f"rd{p}_{qb}")
        nc.sync.dma_start(out=rd, in_=rr)
        rbc = sm_pool.tile([D, 2, 512], F16, tag="rbc", name=f"rbc{p}_{qb}")
        nc.sync.dma_start(out=rbc, in_=rd.to_broadcast((D, 2, 512)))
        nc.gpsimd.tensor_mul(attnT_sb[0:D, p, qs], oc[0:D, 0, :], rbc[:, 0, :])
        stg = sm_pool.tile([D, 512], F16, tag="stg", name=f"stg{p}_{qb}")
        nc.gpsimd.tensor_mul(stg, oc[0:D, 1, :], rbc[:, 1, :])
        nc.sync.dma_start(out=attnT_sb[D:128, p, qs], in_=stg)

    # --- output projection: out tile = attnT.T @ wo ---
    def emit_wo_part(qts):
        for qt in qts:
            o_sb = ob_pool.tile([128, 1024], F16, tag="ob", name=f"ob{qt}")
            for nt in range(2):
                ps = mm_ps.tile([128, 512], F32, tag="mm", name=f"ops{qt}_{nt}")
                for p in range(2):
                    nc.tensor.matmul(
                        ps,
                        lhsT=attnT_sb[:, p, qt * 128 : (qt + 1) * 128],
                        rhs=wo_sb[:, p, nt * 512 : (nt + 1) * 512],
                        start=(p == 0),
                        stop=(p == 1),
                    )
                nc.vector.tensor_copy(o_sb[:, nt * 512 : (nt + 1) * 512], ps)
            nc.sync.dma_start(out=out[qt * 128 : (qt + 1) * 128, :], in_=o_sb)

    def on(ph):
        return phases is None or ph in phases

    if phases is not None:
        # bisection mode: simple phase ordering
        if on("qk"):
            emit_qk(0)
        if on("v"):
            emit_v()
        if on("attn"):
            for qb in range(NQB):
                emit_attn(0, qb)
        if on("qk"):
            emit_qk(1)
        if on("attn"):
            for qb in range(NQB):
                emit_attn(1, qb)
        if on("wo"):
            emit_wo_part(range(NKT))
    else:
        # pipelined ordering: V and pair-1 QK hide under pair-0 attention,
        # Wo hides under pair-1 attention (shifted one block for the
        # normalization DRAM-bounce latency)
        for qb in range(NQB):
            emit_qk_part(0, qb)
            emit_v(range(4 * qb, 4 * qb + 4))
            emit_attn(0, qb)
            emit_qk_part(1, qb)
        # pair-1 block order ends on the smallest (qb=0) block so the final
        # normalize->Wo chain is short; each wo part trails by one block to
        # hide the normalization DRAM-bounce latency
        for qb, wo_qts in ((1, None), (2, range(4, 8)), (3, range(8, 12)),
                           (0, range(12, 16))):
            emit_attn(1, qb)
            if wo_qts is not None:
                emit_wo_part(wo_qts)
        emit_wo_part(range(0, 4))
    if dbg:
        for name, sb in (
            ("d_qt", qt_sb), ("d_kt", kt_sb), ("d_v", v_sb), ("d_at", attnT_sb)
        ):
            if name in dbg:
                nc.sync.dma_start(out=dbg[name], in_=sb)
    ctx.close()


def make_masks():
    i = np.arange(128)[:, None]
    j = np.arange(512)[None, :]
    m = np.stack([(i + 128 * r <= j) for r in range(4)], axis=0).astype(np.float16)
    return np.concatenate([m, m], axis=2)  # duplicated per head pair


def make_in_maps(x, Wq, bq, Wk, bk, Wv, bv, Wo):
    masks = make_masks()
    in_maps = []
    xTb = [np.ascontiguousarray(x[b].T.astype(np.float16)) for b in range(2)]
    for c in range(8):
        b, hg = divmod(c, 4)
        sl = slice(hg * S, (hg + 1) * S)
        in_maps.append(
            {
                "xT": xTb[b],
                "wq": np.ascontiguousarray(Wq[:, sl].astype(np.float16)),
                "wk": np.ascontiguousarray(Wk[:, sl].astype(np.float16)),
                "wv": np.ascontiguousarray(Wv[:, sl].astype(np.float16)),
                "wo": np.ascontiguousarray(Wo[sl, :].astype(np.float16)),
                "bq": np.ascontiguousarray(bq[sl].astype(np.float32).reshape(S, 1)),
                "bk": np.ascontiguousarray(bk[sl].astype(np.float32).reshape(S, 1)),
                "bv": np.ascontiguousarray(bv[sl].astype(np.float32).reshape(1, S)),
                "masks": masks,
            }
        )
    return in_maps


_NC_CACHE = None


def _get_nc():
    global _NC_CACHE
    if _NC_CACHE is None:
        _NC_CACHE = build_nc()
    return _NC_CACHE


def _run(x, Wq, bq, Wk, bk, Wv, bv, Wo, bo, trace=False, **spmd_kwargs):
    nc = _get_nc()
    in_maps = make_in_maps(
        np.asarray(x), np.asarray(Wq), np.asarray(bq), np.asarray(Wk),
        np.asarray(bk), np.asarray(Wv), np.asarray(bv), np.asarray(Wo),
    )
    res = run_bass_kernel_spmd(
        nc, in_maps, core_ids=list(range(8)), trace=trace, **spmd_kwargs
    )
    out = np.zeros((2, T, E), dtype=np.float32)
    for c in range(8):
        out[c // 4] += res.results[c]["out"]
    out += np.asarray(bo, dtype=np.float32)[None, None, :]
    return out, res


def kernel(x, Wq, bq, Wk, bk, Wv, bv, Wo, bo):
    out, _ = _run(x, Wq, bq, Wk, bk, Wv, bv, Wo, bo)
    return out



# revision 43
# speedup vs baseline: 1.0911x; 1.0911x over previous
"""Causal self-attention (B=2, T=2048, E=1024, H=16, D=64) on 8 TRN2 NeuronCores.

Sharding: core = (batch b, head-group hg): 2 batches x 4 head-groups of 4 heads.
Each core computes QKV projections for its 4 heads (256 columns), causal
attention, and the output projection against its 256 rows of Wo, producing a
partial [2048, 1024] output. Host sums the 4 head-group partials per batch
(the tensor-parallel all-reduce) and adds bo.

Precision/performance structure (matmul cost goes by output columns; fp8e4
DoubleRow pairs two K-tiles per instruction at half cost):
  - QKV projections in fp8 hi+lo split precision: W' = 32*W and x decompose
    as hi8 + lo8 host-side; (whi+wlo)@x_hi + whi@x_lo runs as DoubleRow pairs
    for 0.75x the fp16 cost at ~fp16 accuracy. The 32x weight scale keeps lo
    residuals above e4m3's subnormal floor; it is folded back via the exp
    scale and V's 32.0 rowsum column.
  - Scores per head pair: diagonal-block tiles fp16; strictly-lower tiles use
    fp8 q/k via a zero-padded DoubleRow (half cost). Lower-tile probs
    quantize to fp8 and feed DoubleRow AV against V's hi8+lo8 split (half
    cost); diagonal tiles stay fp16 (multiplicative causal mask), so every
    query's nearest <=512 keys are full precision - far-key quantization
    noise averages out in the softmax.
  - Emission interleaves independent matmul "filler" units (V tiles, next
    QK parts, output-projection tiles) into the attention instruction stream:
    the PE queue is in-order, so without fillers it head-of-line blocks on
    ScalarE exp between score and AV instructions.
"""
from contextlib import ExitStack

import numpy as np
import ml_dtypes

import concourse.bass as bass  # noqa: F401
import concourse.mybir as mybir
import concourse.tile as tile
from concourse import bacc
from concourse.bass_utils import run_bass_kernel_spmd

T = 2048
E = 1024
HPC = 4          # heads per core
D = 64
S = HPC * D      # 256: per-core head-column slice
KE = E // 128    # 8 contraction tiles for the projections
NKT = T // 128   # 16 key row tiles
NQB = T // 512   # 4 query column blocks
VP = 96          # padded V columns for DoubleRow AV (64 vals + rowsum + pad)
SC = 32.0        # weight scale: keeps fp8 lo residuals out of subnormals
ES = 0.125 / (SC * SC)  # exp scale with the q/k scale folded in
F8 = mybir.dt.float8e4
F16 = mybir.dt.float16
F32 = mybir.dt.float32
EXP = mybir.ActivationFunctionType.Exp
DR = mybir.MatmulPerfMode.DoubleRow
NP8 = ml_dtypes.float8_e4m3fn


def build_nc(phases=None):
    nc = bacc.Bacc("TRN2", target_bir_lowering=False, debug=False)
    x8 = nc.dram_tensor("x8", [E, 2, T], F8, kind="ExternalInput").ap()
    wqh = nc.dram_tensor("wqh", [E, 2, S], F8, kind="ExternalInput").ap()
    wkh = nc.dram_tensor("wkh", [E, 2, S], F8, kind="ExternalInput").ap()
    wvh = nc.dram_tensor("wvh", [E, 2, S], F8, kind="ExternalInput").ap()
    wql = nc.dram_tensor("wql", [E, S], F8, kind="ExternalInput").ap()
    wkl = nc.dram_tensor("wkl", [E, S], F8, kind="ExternalInput").ap()
    wvl = nc.dram_tensor("wvl", [E, S], F8, kind="ExternalInput").ap()
    wo = nc.dram_tensor("wo", [S, E], F16, kind="ExternalInput").ap()
    bq = nc.dram_tensor("bq", [S, 1], F32, kind="ExternalInput").ap()
    bk = nc.dram_tensor("bk", [S, 1], F32, kind="ExternalInput").ap()
    bv = nc.dram_tensor("bv", [1, S], F32, kind="ExternalInput").ap()
    masks = nc.dram_tensor("masks", [4, 128, 1024], F16, kind="ExternalInput").ap()
    out = nc.dram_tensor("out", [T, E], F16, kind="ExternalOutput").ap()

    with tile.TileContext(nc) as tc:
        _emit(nc, tc, x8, (wqh, wkh, wvh), (wql, wkl, wvl), wo, bq, bk, bv,
              masks, out, phases=phases)
    nc.compile()
    return nc


def _emit(nc, tc, x8, whs, wls, wo, bq, bk, bv, masks, out, phases=None):
    wqh, wkh, wvh = whs
    wql, wkl, wvl = wls
    ctx = ExitStack()
    consts = ctx.enter_context(tc.tile_pool(name="consts", bufs=1))
    mm_ps = ctx.enter_context(tc.tile_pool(name="mm_ps", bufs=2, space="PSUM"))
    st_ps = ctx.enter_context(tc.tile_pool(name="st_ps", bufs=2, space="PSUM"))
    ot_ps = ctx.enter_context(tc.tile_pool(name="ot_ps", bufs=2, space="PSUM"))
    pt_pool = ctx.enter_context(tc.tile_pool(name="pt", bufs=4))
    p8_pool = ctx.enter_context(tc.tile_pool(name="p8", bufs=4))
    sm_pool = ctx.enter_context(tc.tile_pool(name="sm", bufs=4))
    ob_pool = ctx.enter_context(tc.tile_pool(name="ob", bufs=4))
    dr_pool = ctx.enter_context(tc.tile_pool(name="dr", bufs=8, space="DRAM"))

    # --- constant tiles ---
    x_sb = consts.tile([128, KE, 2, T], F8)       # (hi, lo) of x^T
    wqh_sb = consts.tile([128, KE, 2, S], F8)     # duplicated hi weights
    wkh_sb = consts.tile([128, KE, 2, S], F8)
    wvh_sb = consts.tile([128, KE, 2, S], F8)
    wql_sb = consts.tile([128, KE, S], F8)        # lo weights
    wkl_sb = consts.tile([128, KE, S], F8)
    wvl_sb = consts.tile([128, KE, S], F8)
    wo_sb = consts.tile([128, S // 128, E], F16)
    bq_sb = consts.tile([128, 2], F32)
    bk_sb = consts.tile([128, 2], F32)
    bv_bc = consts.tile([128, S], F32)
    mask_sb = consts.tile([128, 4, 1024], F16)
    qt_sb = consts.tile([128, 2, T], F16)
    kt_sb = consts.tile([128, 2, T], F16)
    q8_sb = consts.tile([128, 2, 2, T], F8)       # fp8 q, slot 1 zero-padded
    k8_sb = consts.tile([128, 2, 2, T], F8)       # fp8 k, slot 1 zero-padded
    v_sb = consts.tile([128, NKT, HPC, D + 1], F16)
    v8h_sb = consts.tile([128, NKT, HPC, VP], F8)
    v8l_sb = consts.tile([128, NKT, HPC, VP], F8)
    attnT_sb = consts.tile([128, 2, T], F16)

    # batched constant loads, ordered so the first QK group can start after
    # wq + the first x query-block instead of the full x tensor
    xr = x8.rearrange("(ke p) two t -> p ke two t", p=128)
    wqhr = wqh.rearrange("(ke p) two s -> p ke two s", p=128)
    nc.sync.dma_start(out=wqh_sb[:, 0:4], in_=wqhr[:, 0:4])
    for i in range(2):
        nc.sync.dma_start(out=x_sb[:, 0:4, i, 0:512], in_=xr[:, 0:4, i, 0:512])
    nc.sync.dma_start(out=wqh_sb[:, 4:8], in_=wqhr[:, 4:8])
    for i in range(2):
        nc.sync.dma_start(out=x_sb[:, 4:8, i, 0:512], in_=xr[:, 4:8, i, 0:512])
    # order: everything attn(0,0) needs (q, k, masks, v) before the
    # remaining x query blocks; the start is DMA-serial so order = latency
    nc.sync.dma_start(out=wql_sb, in_=wql.rearrange("(ke p) s -> p ke s", p=128))
    nc.sync.dma_start(out=bq_sb, in_=bq.rearrange("(a p) one -> p (a one)", p=128))
    nc.sync.dma_start(out=wkh_sb, in_=wkh.rearrange("(ke p) two s -> p ke two s", p=128))
    nc.sync.dma_start(out=wkl_sb, in_=wkl.rearrange("(ke p) s -> p ke s", p=128))
    nc.sync.dma_start(out=bk_sb, in_=bk.rearrange("(a p) one -> p (a one)", p=128))
    nc.sync.dma_start(out=mask_sb, in_=masks.rearrange("r p n -> p r n"))
    nc.sync.dma_start(out=wvh_sb, in_=wvh.rearrange("(ke p) two s -> p ke two s", p=128))
    nc.sync.dma_start(out=wvl_sb, in_=wvl.rearrange("(ke p) s -> p ke s", p=128))
    nc.sync.dma_start(out=bv_bc, in_=bv.to_broadcast((128, S)))
    for qb in range(1, NQB):
        qs = slice(qb * 512, (qb + 1) * 512)
        for i in range(2):
            nc.sync.dma_start(out=x_sb[:, :, i, qs], in_=xr[:, :, i, qs])
    nc.sync.dma_start(out=wo_sb, in_=wo.rearrange("(a p) n -> p a n", p=128))
    nc.vector.memset(v_sb[:, :, :, D : D + 1], SC)
    nc.gpsimd.memset(v8h_sb[:, :, :, D : D + 1], SC)
    nc.gpsimd.memset(v8h_sb[:, :, :, D + 1 : VP], 0.0)
    nc.gpsimd.memset(v8l_sb[:, :, :, D : VP], 0.0)
    nc.gpsimd.memset(q8_sb[:, :, 1, :], 0.0)
    nc.gpsimd.memset(k8_sb[:, :, 1, :], 0.0)

    # --- V = x @ wv + bv (fp8 hi-lo DoubleRow), plus fp8 hi/lo splits ---
    def emit_v(rt):
        rsl = slice(rt * 128, (rt + 1) * 128)
        ps = mm_ps.tile([128, 512], F32, tag="mm", name=f"vps{rt}")
        for ke in range(KE):
            nc.tensor.matmul(
                ps[:, 0:S],
                lhsT=x_sb[:, ke, :, rsl],
                rhs=wvh_sb[:, ke],
                start=(ke == 0),
                stop=False,
                perf_mode=DR,
            )
        for a in range(KE // 2):
            nc.tensor.matmul(
                ps[:, 0:S],
                lhsT=x_sb[:, 2 * a : 2 * a + 2, 0, rsl],
                rhs=wvl_sb[:, 2 * a : 2 * a + 2, :],
                start=False,
                stop=(a == KE // 2 - 1),
                perf_mode=DR,
            )
        nc.vector.tensor_add(
            v_sb[:, rt, :, 0:D],
            ps[:, 0:S].rearrange("p (h d) -> p h d", h=HPC),
            bv_bc.rearrange("p (h d) -> p h d", h=HPC),
        )
        nc.gpsimd.tensor_copy(v8h_sb[:, rt, :, 0:D], v_sb[:, rt, :, 0:D])
        nc.gpsimd.tensor_sub(
            v8l_sb[:, rt, :, 0:D], v_sb[:, rt, :, 0:D], v8h_sb[:, rt, :, 0:D]
        )

    # --- QT/KT = (x @ w + b).T for one pair of heads (128 cols) ---
    def emit_qk_part(p, qb, which):
        qs = slice(qb * 512, (qb + 1) * 512)
        psl = slice(p * 128, (p + 1) * 128)
        wh_sb, wl_sb, b_sb, dst, nm = {
            "q": (wqh_sb, wql_sb, bq_sb, qt_sb, "q"),
            "k": (wkh_sb, wkl_sb, bk_sb, kt_sb, "k"),
        }[which]
        ps = mm_ps.tile([128, 512], F32, tag="mm", name=f"{nm}ps{p}_{qb}")
        for ke in range(KE):
            nc.tensor.matmul(
                ps,
                lhsT=wh_sb[:, ke, :, psl],
                rhs=x_sb[:, ke, :, qs],
                start=(ke == 0),
                stop=False,
                perf_mode=DR,
            )
        for a in range(KE // 2):
            nc.tensor.matmul(
                ps,
                lhsT=wl_sb[:, 2 * a : 2 * a + 2, psl],
                rhs=x_sb[:, 2 * a : 2 * a + 2, 0, qs],
                start=False,
                stop=(a == KE // 2 - 1),
                perf_mode=DR,
            )
        nc.vector.tensor_scalar_add(dst[:, p, qs], ps, b_sb[:, p : p + 1])
        # fp8 copies for the zero-padded DoubleRow lower-tile scores:
        # q8 needed for query blocks >=1, k8 for key blocks <=2
        if which == "q" and qb > 0:
            nc.vector.tensor_copy(q8_sb[:, p, 0, qs], dst[:, p, qs])
        if which == "k" and qb < 3:
            nc.gpsimd.tensor_copy(k8_sb[:, p, 0, qs], dst[:, p, qs])

    # --- attention for pair p (heads 2p, 2p+1), query block qb ---
    def emit_attn(p, qb, fill, last=False):
        fill.tick()
        qs = slice(qb * 512, (qb + 1) * 512)
        ots = [
            ot_ps.tile([VP, 512], F32, tag="ot", name=f"ot{p}_{qb}_{i}")
            for i in range(2)
        ]

        def do_st8(kt):
            # fp8 zero-padded DoubleRow score tile (strictly-lower keys)
            st = st_ps.tile([128, 1024], F32, tag="st", name=f"st{p}_{qb}_{kt}")
            for hh in range(2):
                hs = slice(hh * 64, (hh + 1) * 64)
                nc.tensor.matmul(
                    st[:, hh * 512 : (hh + 1) * 512],
                    lhsT=k8_sb[hs, p, :, kt * 128 : (kt + 1) * 128],
                    rhs=q8_sb[hs, p, :, qs],
                    start=True,
                    stop=True,
                    perf_mode=DR,
                )
            return st

        def do_st16(kt, off):
            st = st_ps.tile([128, 1024], F32, tag="st", name=f"st{p}_{qb}_{kt}")
            for hh in range(2):
                hs = slice(hh * 64, (hh + 1) * 64)
                nc.tensor.matmul(
                    st[:, hh * 512 + off : (hh + 1) * 512],
                    lhsT=kt_sb[hs, p, kt * 128 : (kt + 1) * 128],
                    rhs=qt_sb[hs, p, qb * 512 + off : (qb + 1) * 512],
                    start=True,
                    stop=True,
                )
            return st

        # lower key tiles: fp8 probs, DoubleRow AV over (hi, lo) V splits
        for a in range(2 * qb):
            st0 = do_st8(2 * a)
            fill.pop(diag=a >= qb)
            st1 = do_st8(2 * a + 1)
            pt8 = p8_pool.tile([128, 2, 1024], F8, tag="p8", name=f"p8_{p}_{qb}_{a}")
            nc.scalar.activation(pt8[:, 0, :], st0, EXP, scale=ES)
            nc.scalar.activation(pt8[:, 1, :], st1, EXP, scale=ES)
            for hh in range(2):
                rhs8 = pt8[:, :, hh * 512 : (hh + 1) * 512]
                for v8 in (v8h_sb, v8l_sb):
                    nc.tensor.matmul(
                        ots[hh],
                        lhsT=v8[:, 2 * a : 2 * a + 2, 2 * p + hh, :],
                        rhs=rhs8,
                        start=(a == 0 and v8 is v8h_sb),
                        stop=False,
                        perf_mode=DR,
                        skip_group_check=(a != 0 or v8 is not v8h_sb),
                    )
            fill.pop(diag=a >= qb)

        # diagonal tiles: fp16 probs, multiplicative causal mask, trimmed.
        # A full-width matmul opens the accumulation group (qb=0 only) and
        # another closes it.
        if qb == 0:
            diag_order = [(0, 0), (2, 256), (3, 384), (1, 0)]
        else:
            diag_order = [(1, 128), (2, 256), (3, 384), (0, 0)]
        for i, (r, off) in enumerate(diag_order):
            kt = 4 * qb + r
            st = do_st16(kt, off)
            fill.pop(diag=True)
            pt = pt_pool.tile([128, 1024], F16, tag="pt", name=f"pt{p}_{qb}_{kt}")
            if off:
                stv = st.rearrange("p (a n) -> p a n", a=2)
                ptv = pt.rearrange("p (a n) -> p a n", a=2)
                mkv = mask_sb[:, r, :].rearrange("p (a n) -> p a n", a=2)
                nc.scalar.activation(ptv[:, :, off:512], stv[:, :, off:512], EXP, scale=ES)
                nc.vector.tensor_mul(
                    ptv[:, :, off:512], ptv[:, :, off:512], mkv[:, :, off:512]
                )
            else:
                nc.scalar.activation(pt, st, EXP, scale=ES)
                nc.vector.tensor_mul(pt, pt, mask_sb[:, r, :])
            start = qb == 0 and i == 0
            stop = i == len(diag_order) - 1
            for hh in range(2):
                nc.tensor.matmul(
                    ots[hh][0 : D + 1, off:512],
                    lhsT=v_sb[:, kt, 2 * p + hh, :],
                    rhs=pt[:, hh * 512 + off : (hh + 1) * 512],
                    start=start,
                    stop=stop,
                    skip_group_check=not start,
                )
            fill.pop(diag=True)

        # normalization: copy both heads' OT out of PSUM, fp32 reciprocal of
        # the rowsum rows (which carry the 1/32 value-scale: the rowsum
        # column of V is 32.0), one DRAM-bounce broadcast, two gpsimd muls.
        # The last-emitted block's chain is the kernel tail: parallelize its
        # copies/muls across engines (ScalarE and DVE are idle by then).
        oc = sm_pool.tile([D + 1, 2, 512], F32, tag="oc", name=f"oc{p}_{qb}")
        nc.vector.tensor_copy(oc[:, 0, :], ots[0][0 : D + 1, :])
        if last:
            nc.scalar.copy(oc[:, 1, :], ots[1][0 : D + 1, :])
        else:
            nc.vector.tensor_copy(oc[:, 1, :], ots[1][0 : D + 1, :])
        rsq = sm_pool.tile([D, 16], F32, tag="rsq", name=f"rsq{p}_{qb}")
        nc.sync.dma_start(out=rsq, in_=oc[D : D + 1, :, :])
        rr = sm_pool.tile([D, 16], F32, tag="rr", name=f"rr{p}_{qb}")
        nc.vector.reciprocal(rr, rsq)
        rd = dr_pool.tile([1, 2, 512], F32, tag="rd", name=f"rd{p}_{qb}")
        nc.sync.dma_start(out=rd, in_=rr)
        rbc = sm_pool.tile([D, 2, 512], F32, tag="rbc", name=f"rbc{p}_{qb}")
        nc.sync.dma_start(out=rbc, in_=rd.to_broadcast((D, 2, 512)))
        stg = sm_pool.tile([D, 512], F16, tag="stg", name=f"stg{p}_{qb}")
        if last:
            nc.vector.tensor_mul(attnT_sb[0:D, p, qs], oc[0:D, 0, :], rbc[:, 0, :])
        else:
            nc.gpsimd.tensor_mul(attnT_sb[0:D, p, qs], oc[0:D, 0, :], rbc[:, 0, :])
        nc.gpsimd.tensor_mul(stg, oc[0:D, 1, :], rbc[:, 1, :])
        nc.sync.dma_start(out=attnT_sb[D:128, p, qs], in_=stg)

    # --- output projection: out tile = attnT.T @ wo ---
    def emit_wo_part(qt, tail=False):
        o_sb = ob_pool.tile([128, 1024], F16, tag="ob", name=f"ob{qt}")
        for nt in range(2):
            ps = mm_ps.tile([128, 512], F32, tag="mm", name=f"ops{qt}_{nt}")
            for p in range(2):
                nc.tensor.matmul(
                    ps,
                    lhsT=attnT_sb[:, p, qt * 128 : (qt + 1) * 128],
                    rhs=wo_sb[:, p, nt * 512 : (nt + 1) * 512],
                    start=(p == 0),
                    stop=(p == 1),
                )
            # tail tiles split copies DVE/ScalarE (both idle at the end)
            if tail and nt == 1:
                nc.scalar.copy(o_sb[:, nt * 512 : (nt + 1) * 512], ps)
            else:
                nc.vector.tensor_copy(o_sb[:, nt * 512 : (nt + 1) * 512], ps)
        nc.sync.dma_start(out=out[qt * 128 : (qt + 1) * 128, :], in_=o_sb)

    class Fill:
        """FIFO of independent emission units; deps are enforced by the tile
        framework's semaphores, so order only affects performance. "late"
        units (Wo tiles, whose attnT producer chain is freshly emitted) pop
        only once mature - in the diagonal phase one block later, or anywhere
        two blocks later - so they neither head-of-line block the PE queue
        nor burst back-to-back (which would starve ScalarE via the mm-pool/
        DVE copy rotation)."""

        def __init__(self):
            self.q = []
            self.done = set()
            self.block = 0
            self.late_budget = 2

        def tick(self):
            self.block += 1
            self.late_budget = 2

        def add(self, key, fn, late=False):
            if key not in self.done:
                self.q.append((key, fn, self.block if late else -2))

        def pop(self, n=1, diag=False):
            for _ in range(n):
                idx = None
                fresh = False
                for i, (key, fn, birth) in enumerate(self.q):
                    if key in self.done:
                        continue
                    age = self.block - birth
                    if age < 1:
                        continue
                    if age == 1 and not (diag and self.late_budget > 0):
                        continue
                    idx = i
                    fresh = age == 1
                    break
                if idx is None:
                    return
                if fresh:
                    self.late_budget -= 1
                key, fn, _ = self.q.pop(idx)
                self.done.add(key)
                fn()

        def ensure(self, key, fn, late=False):
            if key not in self.done:
                self.done.add(key)
                fn()

        def flush(self):
            for key, fn, _ in list(self.q):
                if key not in self.done:
                    self.done.add(key)
                    fn()
            self.q = []

    fill = Fill()

    def qk_unit(p, qb, which):
        return (("qk", p, qb, which), lambda: emit_qk_part(p, qb, which))

    def v_unit(rt):
        return (("v", rt), lambda: emit_v(rt))

    tail_mode = [False]

    def wo_unit(qt):
        return (("wo", qt), lambda: emit_wo_part(qt, tail=tail_mode[0]))

    if phases is not None:
        # bisection mode: simple phase ordering, no fillers
        nofill = Fill()
        if phases is not None and "qk" in phases or phases is None:
            pass
        if "qk" in phases:
            for qb in range(NQB):
                for w in "qk":
                    emit_qk_part(0, qb, w)
                for w in "qk":
                    emit_qk_part(1, qb, w)
        if "v" in phases:
            for rt in range(NKT):
                emit_v(rt)
        if "attn" in phases:
            for qb in range(NQB):
                emit_attn(0, qb, nofill)
            for qb in range(NQB):
                emit_attn(1, qb, nofill)
        if "wo" in phases:
            for qt in range(NKT):
                emit_wo_part(qt)
    else:
        # pair-0 phase: V tiles, later pair-0 QK parts and early pair-1 QK
        # parts ride as fillers inside the exp-paced attention blocks
        for qb in range(NQB):
            fill.ensure(*qk_unit(0, qb, "q"))
            fill.ensure(*qk_unit(0, qb, "k"))
            for rt in range(4 * qb, 4 * qb + 4):
                fill.ensure(*v_unit(rt))
            if qb < NQB - 1:
                for rt in range(4 * qb + 4, 4 * qb + 8):
                    fill.add(*v_unit(rt))
                fill.add(*qk_unit(0, qb + 1, "q"))
                fill.add(*qk_unit(0, qb + 1, "k"))
            if qb == NQB - 1:
                # qk(1,0) must precede ALL pair-1 blocks: its k8 columns
                # 0:512 feed every pair-1 lower tile
                fill.add(*qk_unit(1, 0, "k"))
                fill.add(*qk_unit(1, 1, "q"))
                fill.add(*qk_unit(1, 1, "k"))
            emit_attn(0, qb, fill)
        # pair-1 phase ordered (1, 2, 0, 3): each block's fillers are later
        # blocks' QK parts plus the Wo tiles unlocked so far ("late": popped
        # only once their attnT normalization has had time to land, rate-
        # capped to avoid bursts that starve ScalarE). Ending on qb=3 (the
        # largest block) absorbs the most Wo work; only its own norm chain
        # and Wo tiles (12-15) remain as the tail. Wo units are emitted
        # strictly after their producing normalization (emission order
        # defines dependency direction in the tile framework).
        fill.flush()
        fill.ensure(*qk_unit(1, 1, "q"))
        fill.ensure(*qk_unit(1, 1, "k"))
        fill.add(*qk_unit(1, 2, "q"))
        fill.add(*qk_unit(1, 2, "k"))
        emit_attn(1, 1, fill)
        fill.ensure(*qk_unit(1, 2, "q"))
        fill.ensure(*qk_unit(1, 2, "k"))
        fill.add(*qk_unit(1, 0, "q"))
        fill.add(*qk_unit(1, 0, "k"))
        fill.add(*qk_unit(1, 3, "q"))
        fill.add(*qk_unit(1, 3, "k"))
        for qt in range(4, 8):
            fill.add(*wo_unit(qt), late=True)
        emit_attn(1, 2, fill)
        fill.ensure(*qk_unit(1, 0, "q"))
        fill.ensure(*qk_unit(1, 0, "k"))
        for qt in range(8, 12):
            fill.add(*wo_unit(qt), late=True)
        emit_attn(1, 0, fill)
        fill.ensure(*qk_unit(1, 3, "q"))
        fill.ensure(*qk_unit(1, 3, "k"))
        for qt in range(0, 4):
            fill.add(*wo_unit(qt), late=True)
        emit_attn(1, 3, fill, last=True)
        tail_mode[0] = True
        fill.flush()
        for qt in range(12, 16):
            emit_wo_part(qt, tail=True)
    ctx.close()


def make_masks():
    i = np.arange(128)[:, None]
    j = np.arange(512)[None, :]
    m = np.stack([(i + 128 * r <= j) for r in range(4)], axis=0).astype(np.float16)
    return np.concatenate([m, m], axis=2)  # duplicated per head pair


def _hilo(a):
    hi = a.astype(NP8)
    lo = (a - hi.astype(np.float32)).astype(NP8)
    return hi, lo


def make_in_maps(x, Wq, bq, Wk, bk, Wv, bv, Wo):
    masks = make_masks()
    in_maps = []
    x8b = []
    for b in range(2):
        xT = np.ascontiguousarray(x[b].T.astype(np.float32))
        xh, xl = _hilo(xT)
        x8b.append(np.ascontiguousarray(np.stack([xh, xl], axis=1)))
    wsplits = {}
    for nm, W in (("q", Wq), ("k", Wk), ("v", Wv)):
        for hg in range(4):
            sl = slice(hg * S, (hg + 1) * S)
            wh, wl = _hilo(np.asarray(W[:, sl], np.float32) * SC)
            whd = np.ascontiguousarray(np.stack([wh, wh], axis=1))
            wsplits[(nm, hg)] = (whd, np.ascontiguousarray(wl))
    for c in range(8):
        b, hg = divmod(c, 4)
        sl = slice(hg * S, (hg + 1) * S)
        wqh_, wql_ = wsplits[("q", hg)]
        wkh_, wkl_ = wsplits[("k", hg)]
        wvh_, wvl_ = wsplits[("v", hg)]
        in_maps.append(
            {
                "x8": x8b[b],
                "wqh": wqh_, "wql": wql_,
                "wkh": wkh_, "wkl": wkl_,
                "wvh": wvh_, "wvl": wvl_,
                "wo": np.ascontiguousarray(Wo[sl, :].astype(np.float16)),
                "bq": np.ascontiguousarray((SC * bq[sl]).astype(np.float32).reshape(S, 1)),
                "bk": np.ascontiguousarray((SC * bk[sl]).astype(np.float32).reshape(S, 1)),
                "bv": np.ascontiguousarray((SC * bv[sl]).astype(np.float32).reshape(1, S)),
                "masks": masks,
            }
        )
    return in_maps


_NC_CACHE = None


def _get_nc():
    global _NC_CACHE
    if _NC_CACHE is None:
        _NC_CACHE = build_nc()
    return _NC_CACHE


def _run(x, Wq, bq, Wk, bk, Wv, bv, Wo, bo, trace=False, **spmd_kwargs):
    nc = _get_nc()
    in_maps = make_in_maps(
        np.asarray(x), np.asarray(Wq), np.asarray(bq), np.asarray(Wk),
        np.asarray(bk), np.asarray(Wv), np.asarray(bv), np.asarray(Wo),
    )
    res = run_bass_kernel_spmd(
        nc, in_maps, core_ids=list(range(8)), trace=trace, **spmd_kwargs
    )
    out = np.zeros((2, T, E), dtype=np.float32)
    for c in range(8):
        out[c // 4] += res.results[c]["out"]
    out += np.asarray(bo, dtype=np.float32)[None, None, :]
    return out, res


def kernel(x, Wq, bq, Wk, bk, Wv, bv, Wo, bo):
    out, _ = _run(x, Wq, bq, Wk, bk, Wv, bv, Wo, bo)
    return out


# revision 44
# speedup vs baseline: 1.0937x; 1.0023x over previous
"""Causal self-attention (B=2, T=2048, E=1024, H=16, D=64) on 8 TRN2 NeuronCores.

Sharding: core = (batch b, head-group hg): 2 batches x 4 head-groups of 4 heads.
Each core computes QKV projections for its 4 heads (256 columns), causal
attention, and the output projection against its 256 rows of Wo, producing a
partial [2048, 1024] output. Host sums the 4 head-group partials per batch
(the tensor-parallel all-reduce) and adds bo.

Precision/performance structure (matmul cost goes by output columns; fp8e4
DoubleRow pairs two K-tiles per instruction at half cost):
  - QKV projections in fp8 hi+lo split precision: W' = 32*W and x decompose
    as hi8 + lo8 host-side; (whi+wlo)@x_hi + whi@x_lo runs as DoubleRow pairs
    for 0.75x the fp16 cost at ~fp16 accuracy. The 32x weight scale keeps lo
    residuals above e4m3's subnormal floor; it is folded back via the exp
    scale and V's 32.0 rowsum column.
  - Scores per head pair: diagonal-block tiles fp16; strictly-lower tiles use
    fp8 q/k via a zero-padded DoubleRow (half cost). Lower-tile probs
    quantize to fp8 and feed DoubleRow AV against V's hi8+lo8 split (half
    cost); diagonal tiles stay fp16 (multiplicative causal mask), so every
    query's nearest <=512 keys are full precision - far-key quantization
    noise averages out in the softmax.
  - Emission interleaves independent matmul "filler" units (V tiles, next
    QK parts, output-projection tiles) into the attention instruction stream:
    the PE queue is in-order, so without fillers it head-of-line blocks on
    ScalarE exp between score and AV instructions.
"""
from contextlib import ExitStack

import numpy as np
import ml_dtypes

import concourse.bass as bass  # noqa: F401
import concourse.mybir as mybir
import concourse.tile as tile
from concourse import bacc
from concourse.bass_utils import run_bass_kernel_spmd

T = 2048
E = 1024
HPC = 4          # heads per core
D = 64
S = HPC * D      # 256: per-core head-column slice
KE = E // 128    # 8 contraction tiles for the projections
NKT = T // 128   # 16 key row tiles
NQB = T // 512   # 4 query column blocks
VP = 96          # padded V columns for DoubleRow AV (64 vals + rowsum + pad)
SC = 32.0        # weight scale: keeps fp8 lo residuals out of subnormals
ES = 0.125 / (SC * SC)  # exp scale with the q/k scale folded in
F8 = mybir.dt.float8e4
F16 = mybir.dt.float16
F32 = mybir.dt.float32
EXP = mybir.ActivationFunctionType.Exp
DR = mybir.MatmulPerfMode.DoubleRow
NP8 = ml_dtypes.float8_e4m3fn


def build_nc(phases=None):
    nc = bacc.Bacc("TRN2", target_bir_lowering=False, debug=False)
    x8 = nc.dram_tensor("x8", [E, 2, T], F8, kind="ExternalInput").ap()
    wqh = nc.dram_tensor("wqh", [E, 2, S], F8, kind="ExternalInput").ap()
    wkh = nc.dram_tensor("wkh", [E, 2, S], F8, kind="ExternalInput").ap()
    wvh = nc.dram_tensor("wvh", [E, 2, S], F8, kind="ExternalInput").ap()
    wql = nc.dram_tensor("wql", [E, S], F8, kind="ExternalInput").ap()
    wkl = nc.dram_tensor("wkl", [E, S], F8, kind="ExternalInput").ap()
    wvl = nc.dram_tensor("wvl", [E, S], F8, kind="ExternalInput").ap()
    wo = nc.dram_tensor("wo", [S, E], F16, kind="ExternalInput").ap()
    bq = nc.dram_tensor("bq", [S, 1], F32, kind="ExternalInput").ap()
    bk = nc.dram_tensor("bk", [S, 1], F32, kind="ExternalInput").ap()
    bv = nc.dram_tensor("bv", [1, S], F32, kind="ExternalInput").ap()
    masks = nc.dram_tensor("masks", [4, 128, 1024], F16, kind="ExternalInput").ap()
    out = nc.dram_tensor("out", [T, E], F16, kind="ExternalOutput").ap()

    with tile.TileContext(nc) as tc:
        _emit(nc, tc, x8, (wqh, wkh, wvh), (wql, wkl, wvl), wo, bq, bk, bv,
              masks, out, phases=phases)
    nc.compile()
    return nc


def _emit(nc, tc, x8, whs, wls, wo, bq, bk, bv, masks, out, phases=None):
    wqh, wkh, wvh = whs
    wql, wkl, wvl = wls
    ctx = ExitStack()
    consts = ctx.enter_context(tc.tile_pool(name="consts", bufs=1))
    mm_ps = ctx.enter_context(tc.tile_pool(name="mm_ps", bufs=2, space="PSUM"))
    st_ps = ctx.enter_context(tc.tile_pool(name="st_ps", bufs=2, space="PSUM"))
    ot_ps = ctx.enter_context(tc.tile_pool(name="ot_ps", bufs=2, space="PSUM"))
    pt_pool = ctx.enter_context(tc.tile_pool(name="pt", bufs=4))
    p8_pool = ctx.enter_context(tc.tile_pool(name="p8", bufs=4))
    sm_pool = ctx.enter_context(tc.tile_pool(name="sm", bufs=4))
    ob_pool = ctx.enter_context(tc.tile_pool(name="ob", bufs=4))
    dr_pool = ctx.enter_context(tc.tile_pool(name="dr", bufs=8, space="DRAM"))

    # --- constant tiles ---
    x_sb = consts.tile([128, KE, 2, T], F8)       # (hi, lo) of x^T
    wqh_sb = consts.tile([128, KE, 2, S], F8)     # duplicated hi weights
    wkh_sb = consts.tile([128, KE, 2, S], F8)
    wvh_sb = consts.tile([128, KE, 2, S], F8)
    wql_sb = consts.tile([128, KE, S], F8)        # lo weights
    wkl_sb = consts.tile([128, KE, S], F8)
    wvl_sb = consts.tile([128, KE, S], F8)
    wo_sb = consts.tile([128, S // 128, E], F16)
    bq_sb = consts.tile([128, 2], F32)
    bk_sb = consts.tile([128, 2], F32)
    bv_bc = consts.tile([128, S], F32)
    mask_sb = consts.tile([128, 4, 1024], F16)
    qt_sb = consts.tile([128, 2, T], F16)
    kt_sb = consts.tile([128, 2, T], F16)
    q8_sb = consts.tile([128, 2, 2, T], F8)       # fp8 q, slot 1 zero-padded
    k8_sb = consts.tile([128, 2, 2, T], F8)       # fp8 k, slot 1 zero-padded
    v_sb = consts.tile([128, NKT, HPC, D + 1], F16)
    v8h_sb = consts.tile([128, NKT, HPC, VP], F8)
    v8l_sb = consts.tile([128, NKT, HPC, VP], F8)
    attnT_sb = consts.tile([128, 2, T], F16)

    # batched constant loads, ordered so the first QK group can start after
    # wq + the first x query-block instead of the full x tensor
    xr = x8.rearrange("(ke p) two t -> p ke two t", p=128)
    wqhr = wqh.rearrange("(ke p) two s -> p ke two s", p=128)
    nc.sync.dma_start(out=wqh_sb[:, 0:4], in_=wqhr[:, 0:4])
    for i in range(2):
        nc.sync.dma_start(out=x_sb[:, 0:4, i, 0:512], in_=xr[:, 0:4, i, 0:512])
    nc.sync.dma_start(out=wqh_sb[:, 4:8], in_=wqhr[:, 4:8])
    for i in range(2):
        nc.sync.dma_start(out=x_sb[:, 4:8, i, 0:512], in_=xr[:, 4:8, i, 0:512])
    # order: everything attn(0,0) needs (q, k, masks, v) before the
    # remaining x query blocks; the start is DMA-serial so order = latency
    nc.sync.dma_start(out=wql_sb, in_=wql.rearrange("(ke p) s -> p ke s", p=128))
    nc.sync.dma_start(out=bq_sb, in_=bq.rearrange("(a p) one -> p (a one)", p=128))
    nc.sync.dma_start(out=wkh_sb, in_=wkh.rearrange("(ke p) two s -> p ke two s", p=128))
    nc.sync.dma_start(out=wkl_sb, in_=wkl.rearrange("(ke p) s -> p ke s", p=128))
    nc.sync.dma_start(out=bk_sb, in_=bk.rearrange("(a p) one -> p (a one)", p=128))
    nc.sync.dma_start(out=mask_sb, in_=masks.rearrange("r p n -> p r n"))
    nc.sync.dma_start(out=wvh_sb, in_=wvh.rearrange("(ke p) two s -> p ke two s", p=128))
    nc.sync.dma_start(out=wvl_sb, in_=wvl.rearrange("(ke p) s -> p ke s", p=128))
    nc.sync.dma_start(out=bv_bc, in_=bv.to_broadcast((128, S)))
    for qb in range(1, NQB):
        qs = slice(qb * 512, (qb + 1) * 512)
        for i in range(2):
            nc.sync.dma_start(out=x_sb[:, :, i, qs], in_=xr[:, :, i, qs])
    nc.sync.dma_start(out=wo_sb, in_=wo.rearrange("(a p) n -> p a n", p=128))
    nc.vector.memset(v_sb[:, :, :, D : D + 1], SC)
    # warm the exp activation table during the initial DMA wait
    warm = consts.tile([1, 1], F32)
    nc.vector.memset(warm, 0.0)
    nc.scalar.activation(warm, warm, EXP)

    def emit_presets():
        # zero pads/slots read by the DoubleRow matmuls; not needed until the
        # first lower-tile block, so these sit after attn(0,0) in Pool's
        # queue (they are disjoint from the slot-0/value writes)
        nc.gpsimd.memset(v8h_sb[:, :, :, D : D + 1], SC)
        nc.gpsimd.memset(v8h_sb[:, :, :, D + 1 : VP], 0.0)
        nc.gpsimd.memset(v8l_sb[:, :, :, D : VP], 0.0)
        nc.gpsimd.memset(q8_sb[:, :, 1, :], 0.0)
        nc.gpsimd.memset(k8_sb[:, :, 1, :], 0.0)

    # --- V = x @ wv + bv (fp8 hi-lo DoubleRow), plus fp8 hi/lo splits ---
    def emit_v(rt):
        rsl = slice(rt * 128, (rt + 1) * 128)
        ps = mm_ps.tile([128, 512], F32, tag="mm", name=f"vps{rt}")
        for ke in range(KE):
            nc.tensor.matmul(
                ps[:, 0:S],
                lhsT=x_sb[:, ke, :, rsl],
                rhs=wvh_sb[:, ke],
                start=(ke == 0),
                stop=False,
                perf_mode=DR,
            )
        for a in range(KE // 2):
            nc.tensor.matmul(
                ps[:, 0:S],
                lhsT=x_sb[:, 2 * a : 2 * a + 2, 0, rsl],
                rhs=wvl_sb[:, 2 * a : 2 * a + 2, :],
                start=False,
                stop=(a == KE // 2 - 1),
                perf_mode=DR,
            )
        nc.vector.tensor_add(
            v_sb[:, rt, :, 0:D],
            ps[:, 0:S].rearrange("p (h d) -> p h d", h=HPC),
            bv_bc.rearrange("p (h d) -> p h d", h=HPC),
        )
        nc.gpsimd.tensor_copy(v8h_sb[:, rt, :, 0:D], v_sb[:, rt, :, 0:D])
        nc.gpsimd.tensor_sub(
            v8l_sb[:, rt, :, 0:D], v_sb[:, rt, :, 0:D], v8h_sb[:, rt, :, 0:D]
        )

    # --- QT/KT = (x @ w + b).T for one pair of heads (128 cols) ---
    def emit_qk_part(p, qb, which):
        qs = slice(qb * 512, (qb + 1) * 512)
        psl = slice(p * 128, (p + 1) * 128)
        wh_sb, wl_sb, b_sb, dst, nm = {
            "q": (wqh_sb, wql_sb, bq_sb, qt_sb, "q"),
            "k": (wkh_sb, wkl_sb, bk_sb, kt_sb, "k"),
        }[which]
        ps = mm_ps.tile([128, 512], F32, tag="mm", name=f"{nm}ps{p}_{qb}")
        for ke in range(KE):
            nc.tensor.matmul(
                ps,
                lhsT=wh_sb[:, ke, :, psl],
                rhs=x_sb[:, ke, :, qs],
                start=(ke == 0),
                stop=False,
                perf_mode=DR,
            )
        for a in range(KE // 2):
            nc.tensor.matmul(
                ps,
                lhsT=wl_sb[:, 2 * a : 2 * a + 2, psl],
                rhs=x_sb[:, 2 * a : 2 * a + 2, 0, qs],
                start=False,
                stop=(a == KE // 2 - 1),
                perf_mode=DR,
            )
        nc.vector.tensor_scalar_add(dst[:, p, qs], ps, b_sb[:, p : p + 1])
        # fp8 copies for the zero-padded DoubleRow lower-tile scores:
        # q8 needed for query blocks >=1, k8 for key blocks <=2
        if which == "q" and qb > 0:
            nc.vector.tensor_copy(q8_sb[:, p, 0, qs], dst[:, p, qs])
        if which == "k" and qb < 3:
            nc.gpsimd.tensor_copy(k8_sb[:, p, 0, qs], dst[:, p, qs])

    # --- attention for pair p (heads 2p, 2p+1), query block qb ---
    def emit_attn(p, qb, fill, last=False):
        fill.tick()
        qs = slice(qb * 512, (qb + 1) * 512)
        ots = [
            ot_ps.tile([VP, 512], F32, tag="ot", name=f"ot{p}_{qb}_{i}")
            for i in range(2)
        ]

        def do_st8(kt):
            # fp8 zero-padded DoubleRow score tile (strictly-lower keys)
            st = st_ps.tile([128, 1024], F32, tag="st", name=f"st{p}_{qb}_{kt}")
            for hh in range(2):
                hs = slice(hh * 64, (hh + 1) * 64)
                nc.tensor.matmul(
                    st[:, hh * 512 : (hh + 1) * 512],
                    lhsT=k8_sb[hs, p, :, kt * 128 : (kt + 1) * 128],
                    rhs=q8_sb[hs, p, :, qs],
                    start=True,
                    stop=True,
                    perf_mode=DR,
                )
            return st

        def do_st16(kt, off):
            st = st_ps.tile([128, 1024], F32, tag="st", name=f"st{p}_{qb}_{kt}")
            for hh in range(2):
                hs = slice(hh * 64, (hh + 1) * 64)
                nc.tensor.matmul(
                    st[:, hh * 512 + off : (hh + 1) * 512],
                    lhsT=kt_sb[hs, p, kt * 128 : (kt + 1) * 128],
                    rhs=qt_sb[hs, p, qb * 512 + off : (qb + 1) * 512],
                    start=True,
                    stop=True,
                )
            return st

        # lower key tiles: fp8 probs, DoubleRow AV over (hi, lo) V splits
        for a in range(2 * qb):
            st0 = do_st8(2 * a)
            fill.pop(diag=a >= qb)
            st1 = do_st8(2 * a + 1)
            pt8 = p8_pool.tile([128, 2, 1024], F8, tag="p8", name=f"p8_{p}_{qb}_{a}")
            nc.scalar.activation(pt8[:, 0, :], st0, EXP, scale=ES)
            nc.scalar.activation(pt8[:, 1, :], st1, EXP, scale=ES)
            for hh in range(2):
                rhs8 = pt8[:, :, hh * 512 : (hh + 1) * 512]
                for v8 in (v8h_sb, v8l_sb):
                    nc.tensor.matmul(
                        ots[hh],
                        lhsT=v8[:, 2 * a : 2 * a + 2, 2 * p + hh, :],
                        rhs=rhs8,
                        start=(a == 0 and v8 is v8h_sb),
                        stop=False,
                        perf_mode=DR,
                        skip_group_check=(a != 0 or v8 is not v8h_sb),
                    )
            fill.pop(diag=a >= qb)

        # diagonal tiles: fp16 probs, multiplicative causal mask, trimmed.
        # A full-width matmul opens the accumulation group (qb=0 only) and
        # another closes it.
        if qb == 0:
            diag_order = [(0, 0), (2, 256), (3, 384), (1, 0)]
        else:
            diag_order = [(1, 128), (2, 256), (3, 384), (0, 0)]
        for i, (r, off) in enumerate(diag_order):
            kt = 4 * qb + r
            st = do_st16(kt, off)
            fill.pop(diag=True)
            pt = pt_pool.tile([128, 1024], F16, tag="pt", name=f"pt{p}_{qb}_{kt}")
            if off:
                stv = st.rearrange("p (a n) -> p a n", a=2)
                ptv = pt.rearrange("p (a n) -> p a n", a=2)
                mkv = mask_sb[:, r, :].rearrange("p (a n) -> p a n", a=2)
                nc.scalar.activation(ptv[:, :, off:512], stv[:, :, off:512], EXP, scale=ES)
                nc.vector.tensor_mul(
                    ptv[:, :, off:512], ptv[:, :, off:512], mkv[:, :, off:512]
                )
            else:
                nc.scalar.activation(pt, st, EXP, scale=ES)
                nc.vector.tensor_mul(pt, pt, mask_sb[:, r, :])
            start = qb == 0 and i == 0
            stop = i == len(diag_order) - 1
            for hh in range(2):
                nc.tensor.matmul(
                    ots[hh][0 : D + 1, off:512],
                    lhsT=v_sb[:, kt, 2 * p + hh, :],
                    rhs=pt[:, hh * 512 + off : (hh + 1) * 512],
                    start=start,
                    stop=stop,
                    skip_group_check=not start,
                )
            fill.pop(diag=True)

        # normalization: copy both heads' OT out of PSUM, fp32 reciprocal of
        # the rowsum rows (which carry the 1/32 value-scale: the rowsum
        # column of V is 32.0), one DRAM-bounce broadcast, two gpsimd muls.
        # The last-emitted block's chain is the kernel tail: parallelize its
        # copies/muls across engines (ScalarE and DVE are idle by then).
        oc = sm_pool.tile([D + 1, 2, 512], F32, tag="oc", name=f"oc{p}_{qb}")
        nc.vector.tensor_copy(oc[:, 0, :], ots[0][0 : D + 1, :])
        if last:
            nc.scalar.copy(oc[:, 1, :], ots[1][0 : D + 1, :])
        else:
            nc.vector.tensor_copy(oc[:, 1, :], ots[1][0 : D + 1, :])
        rsq = sm_pool.tile([D, 16], F32, tag="rsq", name=f"rsq{p}_{qb}")
        nc.sync.dma_start(out=rsq, in_=oc[D : D + 1, :, :])
        rr = sm_pool.tile([D, 16], F32, tag="rr", name=f"rr{p}_{qb}")
        nc.vector.reciprocal(rr, rsq)
        rd = dr_pool.tile([1, 2, 512], F32, tag="rd", name=f"rd{p}_{qb}")
        nc.sync.dma_start(out=rd, in_=rr)
        rbc = sm_pool.tile([D, 2, 512], F32, tag="rbc", name=f"rbc{p}_{qb}")
        nc.sync.dma_start(out=rbc, in_=rd.to_broadcast((D, 2, 512)))
        stg = sm_pool.tile([D, 512], F16, tag="stg", name=f"stg{p}_{qb}")
        if last:
            nc.vector.tensor_mul(attnT_sb[0:D, p, qs], oc[0:D, 0, :], rbc[:, 0, :])
        else:
            nc.gpsimd.tensor_mul(attnT_sb[0:D, p, qs], oc[0:D, 0, :], rbc[:, 0, :])
        nc.gpsimd.tensor_mul(stg, oc[0:D, 1, :], rbc[:, 1, :])
        nc.sync.dma_start(out=attnT_sb[D:128, p, qs], in_=stg)

    # --- output projection: out tile = attnT.T @ wo ---
    def emit_wo_part(qt, tail=False):
        o_sb = ob_pool.tile([128, 1024], F16, tag="ob", name=f"ob{qt}")
        for nt in range(2):
            # in the tail, attention is done: borrow the idle score-PSUM pool
            # for every other tile to double the accumulate/copy rotation
            pool = st_ps if tail and nt == 1 else mm_ps
            tag = "st" if pool is st_ps else "mm"
            ps = pool.tile([128, 512], F32, tag=tag, name=f"ops{qt}_{nt}")
            for p in range(2):
                nc.tensor.matmul(
                    ps,
                    lhsT=attnT_sb[:, p, qt * 128 : (qt + 1) * 128],
                    rhs=wo_sb[:, p, nt * 512 : (nt + 1) * 512],
                    start=(p == 0),
                    stop=(p == 1),
                )
            # tail tiles split copies DVE/ScalarE (both idle at the end)
            if tail and nt == 1:
                nc.scalar.copy(o_sb[:, nt * 512 : (nt + 1) * 512], ps)
            else:
                nc.vector.tensor_copy(o_sb[:, nt * 512 : (nt + 1) * 512], ps)
        nc.sync.dma_start(out=out[qt * 128 : (qt + 1) * 128, :], in_=o_sb)

    class Fill:
        """FIFO of independent emission units; deps are enforced by the tile
        framework's semaphores, so order only affects performance. "late"
        units (Wo tiles, whose attnT producer chain is freshly emitted) pop
        only once mature - in the diagonal phase one block later, or anywhere
        two blocks later - so they neither head-of-line block the PE queue
        nor burst back-to-back (which would starve ScalarE via the mm-pool/
        DVE copy rotation)."""

        def __init__(self):
            self.q = []
            self.done = set()
            self.block = 0
            self.late_budget = 2

        def tick(self):
            self.block += 1
            self.late_budget = 2

        def add(self, key, fn, late=False):
            if key not in self.done:
                self.q.append((key, fn, self.block if late else -2))

        def pop(self, n=1, diag=False):
            for _ in range(n):
                idx = None
                fresh = False
                for i, (key, fn, birth) in enumerate(self.q):
                    if key in self.done:
                        continue
                    age = self.block - birth
                    if age < 1:
                        continue
                    if age == 1 and not (diag and self.late_budget > 0):
                        continue
                    idx = i
                    fresh = age == 1
                    break
                if idx is None:
                    return
                if fresh:
                    self.late_budget -= 1
                key, fn, _ = self.q.pop(idx)
                self.done.add(key)
                fn()

        def ensure(self, key, fn, late=False):
            if key not in self.done:
                self.done.add(key)
                fn()

        def flush(self):
            for key, fn, _ in list(self.q):
                if key not in self.done:
                    self.done.add(key)
                    fn()
            self.q = []

    fill = Fill()

    def qk_unit(p, qb, which):
        return (("qk", p, qb, which), lambda: emit_qk_part(p, qb, which))

    def v_unit(rt):
        return (("v", rt), lambda: emit_v(rt))

    tail_mode = [False]

    def wo_unit(qt):
        return (("wo", qt), lambda: emit_wo_part(qt, tail=tail_mode[0]))

    if phases is not None:
        # bisection mode: simple phase ordering, no fillers
        emit_presets()
        nofill = Fill()
        if phases is not None and "qk" in phases or phases is None:
            pass
        if "qk" in phases:
            for qb in range(NQB):
                for w in "qk":
                    emit_qk_part(0, qb, w)
                for w in "qk":
                    emit_qk_part(1, qb, w)
        if "v" in phases:
            for rt in range(NKT):
                emit_v(rt)
        if "attn" in phases:
            for qb in range(NQB):
                emit_attn(0, qb, nofill)
            for qb in range(NQB):
                emit_attn(1, qb, nofill)
        if "wo" in phases:
            for qt in range(NKT):
                emit_wo_part(qt)
    else:
        # pair-0 phase: V tiles, later pair-0 QK parts and early pair-1 QK
        # parts ride as fillers inside the exp-paced attention blocks
        for qb in range(NQB):
            fill.ensure(*qk_unit(0, qb, "q"))
            fill.ensure(*qk_unit(0, qb, "k"))
            for rt in range(4 * qb, 4 * qb + 4):
                fill.ensure(*v_unit(rt))
            if qb < NQB - 1:
                for rt in range(4 * qb + 4, 4 * qb + 8):
                    fill.add(*v_unit(rt))
                fill.add(*qk_unit(0, qb + 1, "q"))
                fill.add(*qk_unit(0, qb + 1, "k"))
            if qb == NQB - 1:
                # qk(1,0) must precede ALL pair-1 blocks: its k8 columns
                # 0:512 feed every pair-1 lower tile
                fill.add(*qk_unit(1, 0, "k"))
                fill.add(*qk_unit(1, 1, "q"))
                fill.add(*qk_unit(1, 1, "k"))
            emit_attn(0, qb, fill)
            if qb == 0:
                emit_presets()
        # pair-1 phase ordered (1, 2, 0, 3): each block's fillers are later
        # blocks' QK parts plus the Wo tiles unlocked so far ("late": popped
        # only once their attnT normalization has had time to land, rate-
        # capped to avoid bursts that starve ScalarE). Ending on qb=3 (the
        # largest block) absorbs the most Wo work; only its own norm chain
        # and Wo tiles (12-15) remain as the tail. Wo units are emitted
        # strictly after their producing normalization (emission order
        # defines dependency direction in the tile framework).
        fill.flush()
        fill.ensure(*qk_unit(1, 1, "q"))
        fill.ensure(*qk_unit(1, 1, "k"))
        fill.add(*qk_unit(1, 2, "q"))
        fill.add(*qk_unit(1, 2, "k"))
        emit_attn(1, 1, fill)
        fill.ensure(*qk_unit(1, 2, "q"))
        fill.ensure(*qk_unit(1, 2, "k"))
        fill.add(*qk_unit(1, 0, "q"))
        fill.add(*qk_unit(1, 0, "k"))
        fill.add(*qk_unit(1, 3, "q"))
        fill.add(*qk_unit(1, 3, "k"))
        for qt in range(4, 8):
            fill.add(*wo_unit(qt), late=True)
        emit_attn(1, 2, fill)
        fill.ensure(*qk_unit(1, 0, "q"))
        fill.ensure(*qk_unit(1, 0, "k"))
        for qt in range(8, 12):
            fill.add(*wo_unit(qt), late=True)
        emit_attn(1, 0, fill)
        fill.ensure(*qk_unit(1, 3, "q"))
        fill.ensure(*qk_unit(1, 3, "k"))
        for qt in range(0, 4):
            fill.add(*wo_unit(qt), late=True)
        emit_attn(1, 3, fill, last=True)
        tail_mode[0] = True
        fill.flush()
        for qt in range(12, 16):
            emit_wo_part(qt, tail=True)
    ctx.close()


def make_masks():
    i = np.arange(128)[:, None]
    j = np.arange(512)[None, :]
    m = np.stack([(i + 128 * r <= j) for r in range(4)], axis=0).astype(np.float16)
    return np.concatenate([m, m], axis=2)  # duplicated per head pair


def _hilo(a):
    hi = a.astype(NP8)
    lo = (a - hi.astype(np.float32)).astype(NP8)
    return hi, lo


def make_in_maps(x, Wq, bq, Wk, bk, Wv, bv, Wo):
    masks = make_masks()
    in_maps = []
    x8b = []
    for b in range(2):
        xT = np.ascontiguousarray(x[b].T.astype(np.float32))
        xh, xl = _hilo(xT)
        x8b.append(np.ascontiguousarray(np.stack([xh, xl], axis=1)))
    wsplits = {}
    for nm, W in (("q", Wq), ("k", Wk), ("v", Wv)):
        for hg in range(4):
            sl = slice(hg * S, (hg + 1) * S)
            wh, wl = _hilo(np.asarray(W[:, sl], np.float32) * SC)
            whd = np.ascontiguousarray(np.stack([wh, wh], axis=1))
            wsplits[(nm, hg)] = (whd, np.ascontiguousarray(wl))
    for c in range(8):
        b, hg = divmod(c, 4)
        sl = slice(hg * S, (hg + 1) * S)
        wqh_, wql_ = wsplits[("q", hg)]
        wkh_, wkl_ = wsplits[("k", hg)]
        wvh_, wvl_ = wsplits[("v", hg)]
        in_maps.append(
            {
                "x8": x8b[b],
                "wqh": wqh_, "wql": wql_,
                "wkh": wkh_, "wkl": wkl_,
                "wvh": wvh_, "wvl": wvl_,
                "wo": np.ascontiguousarray(Wo[sl, :].astype(np.float16)),
                "bq": np.ascontiguousarray((SC * bq[sl]).astype(np.float32).reshape(S, 1)),
                "bk": np.ascontiguousarray((SC * bk[sl]).astype(np.float32).reshape(S, 1)),
                "bv": np.ascontiguousarray((SC * bv[sl]).astype(np.float32).reshape(1, S)),
                "masks": masks,
            }
        )
    return in_maps


_NC_CACHE = None


def _get_nc():
    global _NC_CACHE
    if _NC_CACHE is None:
        _NC_CACHE = build_nc()
    return _NC_CACHE


def _run(x, Wq, bq, Wk, bk, Wv, bv, Wo, bo, trace=False, **spmd_kwargs):
    nc = _get_nc()
    in_maps = make_in_maps(
        np.asarray(x), np.asarray(Wq), np.asarray(bq), np.asarray(Wk),
        np.asarray(bk), np.asarray(Wv), np.asarray(bv), np.asarray(Wo),
    )
    res = run_bass_kernel_spmd(
        nc, in_maps, core_ids=list(range(8)), trace=trace, **spmd_kwargs
    )
    out = np.zeros((2, T, E), dtype=np.float32)
    for c in range(8):
        out[c // 4] += res.results[c]["out"]
    out += np.asarray(bo, dtype=np.float32)[None, None, :]
    return out, res


def kernel(x, Wq, bq, Wk, bk, Wv, bv, Wo, bo):
    out, _ = _run(x, Wq, bq, Wk, bk, Wv, bv, Wo, bo)
    return out


# revision 52
# speedup vs baseline: 1.0943x; 1.0006x over previous
"""Causal self-attention (B=2, T=2048, E=1024, H=16, D=64) on 8 TRN2 NeuronCores.

Sharding: core = (batch b, head-group hg): 2 batches x 4 head-groups of 4 heads.
Each core computes QKV projections for its 4 heads (256 columns), causal
attention, and the output projection against its 256 rows of Wo, producing a
partial [2048, 1024] output. Host sums the 4 head-group partials per batch
(the tensor-parallel all-reduce) and adds bo.

Precision/performance structure (matmul cost goes by output columns; fp8e4
DoubleRow pairs two K-tiles per instruction at half cost):
  - QKV projections in fp8 hi+lo split precision: W' = 32*W and x decompose
    as hi8 + lo8 host-side; (whi+wlo)@x_hi + whi@x_lo runs as DoubleRow pairs
    for 0.75x the fp16 cost at ~fp16 accuracy. The 32x weight scale keeps lo
    residuals above e4m3's subnormal floor; it is folded back via the exp
    scale and V's 32.0 rowsum column.
  - Scores per head pair: diagonal-block tiles fp16; strictly-lower tiles use
    fp8 q/k via a zero-padded DoubleRow (half cost). Lower-tile probs
    quantize to fp8 and feed DoubleRow AV against V's hi8+lo8 split (half
    cost); diagonal tiles stay fp16 (multiplicative causal mask), so every
    query's nearest <=512 keys are full precision - far-key quantization
    noise averages out in the softmax.
  - Emission interleaves independent matmul "filler" units (V tiles, next
    QK parts, output-projection tiles) into the attention instruction stream:
    the PE queue is in-order, so without fillers it head-of-line blocks on
    ScalarE exp between score and AV instructions.
"""
from contextlib import ExitStack

import numpy as np
import ml_dtypes

import concourse.bass as bass  # noqa: F401
import concourse.mybir as mybir
import concourse.tile as tile
from concourse import bacc
from concourse.bass_utils import run_bass_kernel_spmd

T = 2048
E = 1024
HPC = 4          # heads per core
D = 64
S = HPC * D      # 256: per-core head-column slice
KE = E // 128    # 8 contraction tiles for the projections
NKT = T // 128   # 16 key row tiles
NQB = T // 512   # 4 query column blocks
VP = 96          # padded V columns for DoubleRow AV (64 vals + rowsum + pad)
SC = 32.0        # weight scale: keeps fp8 lo residuals out of subnormals
ES = 0.125 / (SC * SC)  # exp scale with the q/k scale folded in
F8 = mybir.dt.float8e4
F16 = mybir.dt.float16
F32 = mybir.dt.float32
EXP = mybir.ActivationFunctionType.Exp
DR = mybir.MatmulPerfMode.DoubleRow
NP8 = ml_dtypes.float8_e4m3fn


def build_nc(phases=None):
    nc = bacc.Bacc("TRN2", target_bir_lowering=False, debug=False)
    x8 = nc.dram_tensor("x8", [E, 2, T], F8, kind="ExternalInput").ap()
    wqh = nc.dram_tensor("wqh", [E, 2, S], F8, kind="ExternalInput").ap()
    wkh = nc.dram_tensor("wkh", [E, 2, S], F8, kind="ExternalInput").ap()
    wvh = nc.dram_tensor("wvh", [E, 2, S], F8, kind="ExternalInput").ap()
    wql = nc.dram_tensor("wql", [E, S], F8, kind="ExternalInput").ap()
    wkl = nc.dram_tensor("wkl", [E, S], F8, kind="ExternalInput").ap()
    wvl = nc.dram_tensor("wvl", [E, S], F8, kind="ExternalInput").ap()
    wo = nc.dram_tensor("wo", [S, E], F16, kind="ExternalInput").ap()
    bq = nc.dram_tensor("bq", [S, 1], F32, kind="ExternalInput").ap()
    bk = nc.dram_tensor("bk", [S, 1], F32, kind="ExternalInput").ap()
    bv = nc.dram_tensor("bv", [1, S], F32, kind="ExternalInput").ap()
    masks = nc.dram_tensor("masks", [4, 128, 1024], F16, kind="ExternalInput").ap()
    out = nc.dram_tensor("out", [T, E], F16, kind="ExternalOutput").ap()

    with tile.TileContext(nc) as tc:
        _emit(nc, tc, x8, (wqh, wkh, wvh), (wql, wkl, wvl), wo, bq, bk, bv,
              masks, out, phases=phases)
    nc.compile()
    return nc


def _emit(nc, tc, x8, whs, wls, wo, bq, bk, bv, masks, out, phases=None):
    wqh, wkh, wvh = whs
    wql, wkl, wvl = wls
    ctx = ExitStack()
    consts = ctx.enter_context(tc.tile_pool(name="consts", bufs=1))
    mm_ps = ctx.enter_context(tc.tile_pool(name="mm_ps", bufs=2, space="PSUM"))
    st_ps = ctx.enter_context(tc.tile_pool(name="st_ps", bufs=2, space="PSUM"))
    ot_ps = ctx.enter_context(tc.tile_pool(name="ot_ps", bufs=2, space="PSUM"))
    pt_pool = ctx.enter_context(tc.tile_pool(name="pt", bufs=6))
    p8_pool = ctx.enter_context(tc.tile_pool(name="p8", bufs=6))
    sm_pool = ctx.enter_context(tc.tile_pool(name="sm", bufs=4))
    ob_pool = ctx.enter_context(tc.tile_pool(name="ob", bufs=4))
    dr_pool = ctx.enter_context(tc.tile_pool(name="dr", bufs=8, space="DRAM"))

    # --- constant tiles ---
    x_sb = consts.tile([128, KE, 2, T], F8)       # (hi, lo) of x^T
    wqh_sb = consts.tile([128, KE, 2, S], F8)     # duplicated hi weights
    wkh_sb = consts.tile([128, KE, 2, S], F8)
    wvh_sb = consts.tile([128, KE, 2, S], F8)
    wql_sb = consts.tile([128, KE, S], F8)        # lo weights
    wkl_sb = consts.tile([128, KE, S], F8)
    wvl_sb = consts.tile([128, KE, S], F8)
    wo_sb = consts.tile([128, S // 128, E], F16)
    bq_sb = consts.tile([128, 2], F32)
    bk_sb = consts.tile([128, 2], F32)
    bv_bc = consts.tile([128, S], F32)
    mask_sb = consts.tile([128, 4, 1024], F16)
    qt_sb = consts.tile([128, 2, T], F16)
    kt_sb = consts.tile([128, 2, T], F16)
    q8_sb = consts.tile([128, 2, 2, T], F8)       # fp8 q, slot 1 zero-padded
    k8_sb = consts.tile([128, 2, 2, T], F8)       # fp8 k, slot 1 zero-padded
    v_sb = consts.tile([128, NKT, HPC, D + 1], F16)
    v8h_sb = consts.tile([128, NKT, HPC, VP], F8)
    v8l_sb = consts.tile([128, NKT, HPC, VP], F8)
    attnT_sb = consts.tile([128, 2, T], F16)

    # batched constant loads, ordered so the first QK group can start after
    # wq + the first x query-block instead of the full x tensor
    xr = x8.rearrange("(ke p) two t -> p ke two t", p=128)
    wqhr = wqh.rearrange("(ke p) two s -> p ke two s", p=128)
    nc.sync.dma_start(out=wqh_sb[:, 0:4], in_=wqhr[:, 0:4])
    for i in range(2):
        nc.sync.dma_start(out=x_sb[:, 0:4, i, 0:512], in_=xr[:, 0:4, i, 0:512])
    nc.sync.dma_start(out=wqh_sb[:, 4:8], in_=wqhr[:, 4:8])
    for i in range(2):
        nc.sync.dma_start(out=x_sb[:, 4:8, i, 0:512], in_=xr[:, 4:8, i, 0:512])
    # order: everything attn(0,0) needs (q, k, masks, v) before the
    # remaining x query blocks; the start is DMA-serial so order = latency
    nc.sync.dma_start(out=wql_sb, in_=wql.rearrange("(ke p) s -> p ke s", p=128))
    nc.sync.dma_start(out=bq_sb, in_=bq.rearrange("(a p) one -> p (a one)", p=128))
    nc.sync.dma_start(out=wkh_sb, in_=wkh.rearrange("(ke p) two s -> p ke two s", p=128))
    nc.sync.dma_start(out=wkl_sb, in_=wkl.rearrange("(ke p) s -> p ke s", p=128))
    nc.sync.dma_start(out=bk_sb, in_=bk.rearrange("(a p) one -> p (a one)", p=128))
    nc.sync.dma_start(out=mask_sb, in_=masks.rearrange("r p n -> p r n"))
    nc.sync.dma_start(out=wvh_sb, in_=wvh.rearrange("(ke p) two s -> p ke two s", p=128))
    nc.sync.dma_start(out=wvl_sb, in_=wvl.rearrange("(ke p) s -> p ke s", p=128))
    nc.sync.dma_start(out=bv_bc, in_=bv.to_broadcast((128, S)))
    for qb in range(1, NQB):
        qs = slice(qb * 512, (qb + 1) * 512)
        for i in range(2):
            nc.sync.dma_start(out=x_sb[:, :, i, qs], in_=xr[:, :, i, qs])
    nc.sync.dma_start(out=wo_sb, in_=wo.rearrange("(a p) n -> p a n", p=128))
    nc.vector.memset(v_sb[:, :, :, D : D + 1], SC)
    # warm the exp activation table during the initial DMA wait
    warm = consts.tile([1, 1], F32)
    nc.vector.memset(warm, 0.0)
    nc.scalar.activation(warm, warm, EXP)

    def emit_presets():
        # zero pads/slots read by the DoubleRow matmuls; not needed until the
        # first lower-tile block, so these sit after attn(0,0) in Pool's
        # queue (they are disjoint from the slot-0/value writes)
        nc.gpsimd.memset(v8h_sb[:, :, :, D : D + 1], SC)
        nc.gpsimd.memset(v8h_sb[:, :, :, D + 1 : VP], 0.0)
        nc.gpsimd.memset(v8l_sb[:, :, :, D : VP], 0.0)
        nc.gpsimd.memset(q8_sb[:, :, 1, :], 0.0)
        nc.gpsimd.memset(k8_sb[:, :, 1, :], 0.0)

    # --- V = x @ wv + bv (fp8 hi-lo DoubleRow), plus fp8 hi/lo splits ---
    def emit_v(rt):
        rsl = slice(rt * 128, (rt + 1) * 128)
        ps = mm_ps.tile([128, 512], F32, tag="mm", name=f"vps{rt}")
        for ke in range(KE):
            nc.tensor.matmul(
                ps[:, 0:S],
                lhsT=x_sb[:, ke, :, rsl],
                rhs=wvh_sb[:, ke],
                start=(ke == 0),
                stop=False,
                perf_mode=DR,
            )
        for a in range(KE // 2):
            nc.tensor.matmul(
                ps[:, 0:S],
                lhsT=x_sb[:, 2 * a : 2 * a + 2, 0, rsl],
                rhs=wvl_sb[:, 2 * a : 2 * a + 2, :],
                start=False,
                stop=(a == KE // 2 - 1),
                perf_mode=DR,
            )
        nc.vector.tensor_add(
            v_sb[:, rt, :, 0:D],
            ps[:, 0:S].rearrange("p (h d) -> p h d", h=HPC),
            bv_bc.rearrange("p (h d) -> p h d", h=HPC),
        )
        nc.gpsimd.tensor_copy(v8h_sb[:, rt, :, 0:D], v_sb[:, rt, :, 0:D])
        nc.gpsimd.tensor_sub(
            v8l_sb[:, rt, :, 0:D], v_sb[:, rt, :, 0:D], v8h_sb[:, rt, :, 0:D]
        )

    # --- QT/KT = (x @ w + b).T for one pair of heads (128 cols) ---
    def emit_qk_part(p, qb, which):
        qs = slice(qb * 512, (qb + 1) * 512)
        psl = slice(p * 128, (p + 1) * 128)
        wh_sb, wl_sb, b_sb, dst, nm = {
            "q": (wqh_sb, wql_sb, bq_sb, qt_sb, "q"),
            "k": (wkh_sb, wkl_sb, bk_sb, kt_sb, "k"),
        }[which]
        ps = mm_ps.tile([128, 512], F32, tag="mm", name=f"{nm}ps{p}_{qb}")
        for ke in range(KE):
            nc.tensor.matmul(
                ps,
                lhsT=wh_sb[:, ke, :, psl],
                rhs=x_sb[:, ke, :, qs],
                start=(ke == 0),
                stop=False,
                perf_mode=DR,
            )
        for a in range(KE // 2):
            nc.tensor.matmul(
                ps,
                lhsT=wl_sb[:, 2 * a : 2 * a + 2, psl],
                rhs=x_sb[:, 2 * a : 2 * a + 2, 0, qs],
                start=False,
                stop=(a == KE // 2 - 1),
                perf_mode=DR,
            )
        nc.vector.tensor_scalar_add(dst[:, p, qs], ps, b_sb[:, p : p + 1])
        # fp8 copies for the zero-padded DoubleRow lower-tile scores:
        # q8 needed for query blocks >=1, k8 for key blocks <=2
        if which == "q" and qb > 0:
            nc.vector.tensor_copy(q8_sb[:, p, 0, qs], dst[:, p, qs])
        if which == "k" and qb < 3:
            nc.gpsimd.tensor_copy(k8_sb[:, p, 0, qs], dst[:, p, qs])

    # --- attention for pair p (heads 2p, 2p+1), query block qb ---
    def emit_attn(p, qb, fill, last=False):
        fill.tick()
        qs = slice(qb * 512, (qb + 1) * 512)
        ots = [
            ot_ps.tile([VP, 512], F32, tag="ot", name=f"ot{p}_{qb}_{i}")
            for i in range(2)
        ]

        def do_st8(kt):
            # fp8 zero-padded DoubleRow score tile (strictly-lower keys)
            st = st_ps.tile([128, 1024], F32, tag="st", name=f"st{p}_{qb}_{kt}")
            for hh in range(2):
                hs = slice(hh * 64, (hh + 1) * 64)
                nc.tensor.matmul(
                    st[:, hh * 512 : (hh + 1) * 512],
                    lhsT=k8_sb[hs, p, :, kt * 128 : (kt + 1) * 128],
                    rhs=q8_sb[hs, p, :, qs],
                    start=True,
                    stop=True,
                    perf_mode=DR,
                )
            return st

        def do_st16(kt, off):
            st = st_ps.tile([128, 1024], F32, tag="st", name=f"st{p}_{qb}_{kt}")
            for hh in range(2):
                hs = slice(hh * 64, (hh + 1) * 64)
                nc.tensor.matmul(
                    st[:, hh * 512 + off : (hh + 1) * 512],
                    lhsT=kt_sb[hs, p, kt * 128 : (kt + 1) * 128],
                    rhs=qt_sb[hs, p, qb * 512 + off : (qb + 1) * 512],
                    start=True,
                    stop=True,
                )
            return st

        # lower key tiles: fp8 probs, DoubleRow AV over (hi, lo) V splits
        for a in range(2 * qb):
            st0 = do_st8(2 * a)
            fill.pop(diag=a >= qb)
            st1 = do_st8(2 * a + 1)
            pt8 = p8_pool.tile([128, 2, 1024], F8, tag="p8", name=f"p8_{p}_{qb}_{a}")
            nc.scalar.activation(pt8[:, 0, :], st0, EXP, scale=ES)
            nc.scalar.activation(pt8[:, 1, :], st1, EXP, scale=ES)
            for hh in range(2):
                rhs8 = pt8[:, :, hh * 512 : (hh + 1) * 512]
                for v8 in (v8h_sb, v8l_sb):
                    nc.tensor.matmul(
                        ots[hh],
                        lhsT=v8[:, 2 * a : 2 * a + 2, 2 * p + hh, :],
                        rhs=rhs8,
                        start=(a == 0 and v8 is v8h_sb),
                        stop=False,
                        perf_mode=DR,
                        skip_group_check=(a != 0 or v8 is not v8h_sb),
                    )
            fill.pop(diag=a >= qb)

        # diagonal tiles: fp16 probs, multiplicative causal mask, trimmed.
        # A full-width matmul opens the accumulation group (qb=0 only) and
        # another closes it.
        if qb == 0:
            diag_order = [(0, 0), (2, 256), (3, 384), (1, 0)]
        else:
            diag_order = [(1, 128), (2, 256), (3, 384), (0, 0)]
        for i, (r, off) in enumerate(diag_order):
            kt = 4 * qb + r
            st = do_st16(kt, off)
            fill.pop(diag=True)
            pt = pt_pool.tile([128, 1024], F16, tag="pt", name=f"pt{p}_{qb}_{kt}")
            if off:
                stv = st.rearrange("p (a n) -> p a n", a=2)
                ptv = pt.rearrange("p (a n) -> p a n", a=2)
                mkv = mask_sb[:, r, :].rearrange("p (a n) -> p a n", a=2)
                nc.scalar.activation(ptv[:, :, off:512], stv[:, :, off:512], EXP, scale=ES)
                nc.vector.tensor_mul(
                    ptv[:, :, off:512], ptv[:, :, off:512], mkv[:, :, off:512]
                )
            else:
                nc.scalar.activation(pt, st, EXP, scale=ES)
                nc.vector.tensor_mul(pt, pt, mask_sb[:, r, :])
            start = qb == 0 and i == 0
            stop = i == len(diag_order) - 1
            for hh in range(2):
                nc.tensor.matmul(
                    ots[hh][0 : D + 1, off:512],
                    lhsT=v_sb[:, kt, 2 * p + hh, :],
                    rhs=pt[:, hh * 512 + off : (hh + 1) * 512],
                    start=start,
                    stop=stop,
                    skip_group_check=not start,
                )
            fill.pop(diag=True)

        # normalization: copy both heads' OT out of PSUM, fp32 reciprocal of
        # the rowsum rows (which carry the 1/32 value-scale: the rowsum
        # column of V is 32.0), one DRAM-bounce broadcast, two gpsimd muls.
        # The last-emitted block's chain is the kernel tail: parallelize its
        # copies/muls across engines (ScalarE and DVE are idle by then).
        oc = sm_pool.tile([D + 1, 2, 512], F32, tag="oc", name=f"oc{p}_{qb}")
        nc.vector.tensor_copy(oc[:, 0, :], ots[0][0 : D + 1, :])
        if last:
            nc.scalar.copy(oc[:, 1, :], ots[1][0 : D + 1, :])
        else:
            nc.vector.tensor_copy(oc[:, 1, :], ots[1][0 : D + 1, :])
        rsq = sm_pool.tile([D, 16], F32, tag="rsq", name=f"rsq{p}_{qb}")
        nc.sync.dma_start(out=rsq, in_=oc[D : D + 1, :, :])
        rr = sm_pool.tile([D, 16], F32, tag="rr", name=f"rr{p}_{qb}")
        nc.vector.reciprocal(rr, rsq)
        rd = dr_pool.tile([1, 2, 512], F32, tag="rd", name=f"rd{p}_{qb}")
        nc.sync.dma_start(out=rd, in_=rr)
        rbc = sm_pool.tile([D, 2, 512], F32, tag="rbc", name=f"rbc{p}_{qb}")
        nc.sync.dma_start(out=rbc, in_=rd.to_broadcast((D, 2, 512)))
        stg = sm_pool.tile([D, 512], F16, tag="stg", name=f"stg{p}_{qb}")
        if last:
            nc.vector.tensor_mul(attnT_sb[0:D, p, qs], oc[0:D, 0, :], rbc[:, 0, :])
        else:
            nc.gpsimd.tensor_mul(attnT_sb[0:D, p, qs], oc[0:D, 0, :], rbc[:, 0, :])
        nc.gpsimd.tensor_mul(stg, oc[0:D, 1, :], rbc[:, 1, :])
        nc.sync.dma_start(out=attnT_sb[D:128, p, qs], in_=stg)

    # --- output projection: out tile = attnT.T @ wo ---
    def emit_wo_part(qt, tail=False):
        o_sb = ob_pool.tile([128, 1024], F16, tag="ob", name=f"ob{qt}")
        for nt in range(2):
            # in the tail, attention is done: borrow the idle score-PSUM pool
            # for every other tile to double the accumulate/copy rotation
            pool = st_ps if tail and nt == 1 else mm_ps
            tag = "st" if pool is st_ps else "mm"
            ps = pool.tile([128, 512], F32, tag=tag, name=f"ops{qt}_{nt}")
            for p in range(2):
                nc.tensor.matmul(
                    ps,
                    lhsT=attnT_sb[:, p, qt * 128 : (qt + 1) * 128],
                    rhs=wo_sb[:, p, nt * 512 : (nt + 1) * 512],
                    start=(p == 0),
                    stop=(p == 1),
                )
            # tail tiles split copies DVE/ScalarE (both idle at the end)
            if tail and nt == 1:
                nc.scalar.copy(o_sb[:, nt * 512 : (nt + 1) * 512], ps)
            else:
                nc.vector.tensor_copy(o_sb[:, nt * 512 : (nt + 1) * 512], ps)
        nc.sync.dma_start(out=out[qt * 128 : (qt + 1) * 128, :], in_=o_sb)

    class Fill:
        """FIFO of independent emission units; deps are enforced by the tile
        framework's semaphores, so order only affects performance. "late"
        units (Wo tiles, whose attnT producer chain is freshly emitted) pop
        only once mature - in the diagonal phase one block later, or anywhere
        two blocks later - so they neither head-of-line block the PE queue
        nor burst back-to-back (which would starve ScalarE via the mm-pool/
        DVE copy rotation)."""

        def __init__(self):
            self.q = []
            self.done = set()
            self.block = 0
            self.late_budget = 2

        def tick(self):
            self.block += 1
            self.late_budget = 2

        def add(self, key, fn, late=False):
            if key not in self.done:
                self.q.append((key, fn, self.block if late else -2))

        def pop(self, n=1, diag=False):
            for _ in range(n):
                idx = None
                fresh = False
                for i, (key, fn, birth) in enumerate(self.q):
                    if key in self.done:
                        continue
                    age = self.block - birth
                    if age < 1:
                        continue
                    if age == 1 and not (diag and self.late_budget > 0):
                        continue
                    idx = i
                    fresh = age == 1
                    break
                if idx is None:
                    return
                if fresh:
                    self.late_budget -= 1
                key, fn, _ = self.q.pop(idx)
                self.done.add(key)
                fn()

        def ensure(self, key, fn, late=False):
            if key not in self.done:
                self.done.add(key)
                fn()

        def flush(self):
            for key, fn, _ in list(self.q):
                if key not in self.done:
                    self.done.add(key)
                    fn()
            self.q = []

    fill = Fill()

    def qk_unit(p, qb, which):
        return (("qk", p, qb, which), lambda: emit_qk_part(p, qb, which))

    def v_unit(rt):
        return (("v", rt), lambda: emit_v(rt))

    tail_mode = [False]

    def wo_unit(qt):
        return (("wo", qt), lambda: emit_wo_part(qt, tail=tail_mode[0]))

    if phases is not None:
        # bisection mode: simple phase ordering, no fillers
        emit_presets()
        nofill = Fill()
        if phases is not None and "qk" in phases or phases is None:
            pass
        if "qk" in phases:
            for qb in range(NQB):
                for w in "qk":
                    emit_qk_part(0, qb, w)
                for w in "qk":
                    emit_qk_part(1, qb, w)
        if "v" in phases:
            for rt in range(NKT):
                emit_v(rt)
        if "attn" in phases:
            for qb in range(NQB):
                emit_attn(0, qb, nofill)
            for qb in range(NQB):
                emit_attn(1, qb, nofill)
        if "wo" in phases:
            for qt in range(NKT):
                emit_wo_part(qt)
    else:
        # pair-0 phase: V tiles, later pair-0 QK parts and early pair-1 QK
        # parts ride as fillers inside the exp-paced attention blocks
        for qb in range(NQB):
            fill.ensure(*qk_unit(0, qb, "q"))
            fill.ensure(*qk_unit(0, qb, "k"))
            for rt in range(4 * qb, 4 * qb + 4):
                fill.ensure(*v_unit(rt))
            if qb < NQB - 1:
                for rt in range(4 * qb + 4, 4 * qb + 8):
                    fill.add(*v_unit(rt))
                fill.add(*qk_unit(0, qb + 1, "q"))
                fill.add(*qk_unit(0, qb + 1, "k"))
            if qb == NQB - 1:
                # qk(1,0) must precede ALL pair-1 blocks: its k8 columns
                # 0:512 feed every pair-1 lower tile
                fill.add(*qk_unit(1, 0, "k"))
                fill.add(*qk_unit(1, 1, "q"))
                fill.add(*qk_unit(1, 1, "k"))
            emit_attn(0, qb, fill)
            if qb == 0:
                emit_presets()
        # pair-1 phase ordered (1, 2, 0, 3): each block's fillers are later
        # blocks' QK parts plus the Wo tiles unlocked so far ("late": popped
        # only once their attnT normalization has had time to land, rate-
        # capped to avoid bursts that starve ScalarE). Ending on qb=3 (the
        # largest block) absorbs the most Wo work; only its own norm chain
        # and Wo tiles (12-15) remain as the tail. Wo units are emitted
        # strictly after their producing normalization (emission order
        # defines dependency direction in the tile framework).
        fill.flush()
        fill.ensure(*qk_unit(1, 1, "q"))
        fill.ensure(*qk_unit(1, 1, "k"))
        fill.add(*qk_unit(1, 2, "q"))
        fill.add(*qk_unit(1, 2, "k"))
        emit_attn(1, 1, fill)
        fill.ensure(*qk_unit(1, 2, "q"))
        fill.ensure(*qk_unit(1, 2, "k"))
        fill.add(*qk_unit(1, 0, "q"))
        fill.add(*qk_unit(1, 0, "k"))
        fill.add(*qk_unit(1, 3, "q"))
        fill.add(*qk_unit(1, 3, "k"))
        for qt in range(4, 8):
            fill.add(*wo_unit(qt), late=True)
        emit_attn(1, 2, fill)
        fill.ensure(*qk_unit(1, 0, "q"))
        fill.ensure(*qk_unit(1, 0, "k"))
        for qt in range(8, 12):
            fill.add(*wo_unit(qt), late=True)
        emit_attn(1, 0, fill)
        fill.ensure(*qk_unit(1, 3, "q"))
        fill.ensure(*qk_unit(1, 3, "k"))
        for qt in range(0, 4):
            fill.add(*wo_unit(qt), late=True)
        emit_attn(1, 3, fill, last=True)
        tail_mode[0] = True
        fill.flush()
        for qt in range(12, 16):
            emit_wo_part(qt, tail=True)
    ctx.close()


def make_masks():
    i = np.arange(128)[:, None]
    j = np.arange(512)[None, :]
    m = np.stack([(i + 128 * r <= j) for r in range(4)], axis=0).astype(np.float16)
    return np.concatenate([m, m], axis=2)  # duplicated per head pair


def _hilo(a):
    hi = a.astype(NP8)
    lo = (a - hi.astype(np.float32)).astype(NP8)
    return hi, lo


def make_in_maps(x, Wq, bq, Wk, bk, Wv, bv, Wo):
    masks = make_masks()
    in_maps = []
    x8b = []
    for b in range(2):
        xT = np.ascontiguousarray(x[b].T.astype(np.float32))
        xh, xl = _hilo(xT)
        x8b.append(np.ascontiguousarray(np.stack([xh, xl], axis=1)))
    wsplits = {}
    for nm, W in (("q", Wq), ("k", Wk), ("v", Wv)):
        for hg in range(4):
            sl = slice(hg * S, (hg + 1) * S)
            wh, wl = _hilo(np.asarray(W[:, sl], np.float32) * SC)
            whd = np.ascontiguousarray(np.stack([wh, wh], axis=1))
            wsplits[(nm, hg)] = (whd, np.ascontiguousarray(wl))
    for c in range(8):
        b, hg = divmod(c, 4)
        sl = slice(hg * S, (hg + 1) * S)
        wqh_, wql_ = wsplits[("q", hg)]
        wkh_, wkl_ = wsplits[("k", hg)]
        wvh_, wvl_ = wsplits[("v", hg)]
        in_maps.append(
            {
                "x8": x8b[b],
                "wqh": wqh_, "wql": wql_,
                "wkh": wkh_, "wkl": wkl_,
                "wvh": wvh_, "wvl": wvl_,
                "wo": np.ascontiguousarray(Wo[sl, :].astype(np.float16)),
                "bq": np.ascontiguousarray((SC * bq[sl]).astype(np.float32).reshape(S, 1)),
                "bk": np.ascontiguousarray((SC * bk[sl]).astype(np.float32).reshape(S, 1)),
                "bv": np.ascontiguousarray((SC * bv[sl]).astype(np.float32).reshape(1, S)),
                "masks": masks,
            }
        )
    return in_maps


_NC_CACHE = None


def _get_nc():
    global _NC_CACHE
    if _NC_CACHE is None:
        _NC_CACHE = build_nc()
    return _NC_CACHE


def _run(x, Wq, bq, Wk, bk, Wv, bv, Wo, bo, trace=False, **spmd_kwargs):
    nc = _get_nc()
    in_maps = make_in_maps(
        np.asarray(x), np.asarray(Wq), np.asarray(bq), np.asarray(Wk),
        np.asarray(bk), np.asarray(Wv), np.asarray(bv), np.asarray(Wo),
    )
    res = run_bass_kernel_spmd(
        nc, in_maps, core_ids=list(range(8)), trace=trace, **spmd_kwargs
    )
    out = np.zeros((2, T, E), dtype=np.float32)
    for c in range(8):
        out[c // 4] += res.results[c]["out"]
    out += np.asarray(bo, dtype=np.float32)[None, None, :]
    return out, res


def kernel(x, Wq, bq, Wk, bk, Wv, bv, Wo, bo):
    out, _ = _run(x, Wq, bq, Wk, bk, Wv, bv, Wo, bo)
    return out


# revision 54
# speedup vs baseline: 1.1126x; 1.0168x over previous
"""Causal self-attention (B=2, T=2048, E=1024, H=16, D=64) on 8 TRN2 NeuronCores.

Sharding: core = (batch b, head-group hg): 2 batches x 4 head-groups of 4 heads.
Each core computes QKV projections for its 4 heads (256 columns), causal
attention, and the output projection against its 256 rows of Wo, producing a
partial [2048, 1024] output. Host sums the 4 head-group partials per batch
(the tensor-parallel all-reduce) and adds bo.

Precision/performance structure (matmul cost goes by output columns; fp8e4
DoubleRow pairs two K-tiles per instruction at half cost):
  - QKV projections in fp8 hi+lo split precision: W' = 32*W and x decompose
    as hi8 + lo8 host-side; (whi+wlo)@x_hi + whi@x_lo runs as DoubleRow pairs
    for 0.75x the fp16 cost at ~fp16 accuracy. The 32x weight scale keeps lo
    residuals above e4m3's subnormal floor; it is folded back via the exp
    scale and V's 32.0 rowsum column.
  - Scores per head pair: diagonal-block tiles fp16; strictly-lower tiles use
    fp8 q/k via a zero-padded DoubleRow (half cost). Lower-tile probs
    quantize to fp8 and feed DoubleRow AV against V's hi8+lo8 split (half
    cost); diagonal tiles stay fp16 (multiplicative causal mask), so every
    query's nearest <=512 keys are full precision - far-key quantization
    noise averages out in the softmax.
  - Emission interleaves independent matmul "filler" units (V tiles, next
    QK parts, output-projection tiles) into the attention instruction stream:
    the PE queue is in-order, so without fillers it head-of-line blocks on
    ScalarE exp between score and AV instructions.
"""
from contextlib import ExitStack

import numpy as np
import ml_dtypes

import concourse.bass as bass  # noqa: F401
import concourse.mybir as mybir
import concourse.tile as tile
from concourse import bacc
from concourse.bass_utils import run_bass_kernel_spmd

T = 2048
E = 1024
HPC = 4          # heads per core
D = 64
S = HPC * D      # 256: per-core head-column slice
KE = E // 128    # 8 contraction tiles for the projections
NKT = T // 128   # 16 key row tiles
NQB = T // 512   # 4 query column blocks
VP = 96          # padded V columns for DoubleRow AV (64 vals + rowsum + pad)
SC = 32.0        # weight scale: keeps fp8 lo residuals out of subnormals
ES = 0.125 / (SC * SC)  # exp scale with the q/k scale folded in
F8 = mybir.dt.float8e4
F16 = mybir.dt.float16
F32 = mybir.dt.float32
EXP = mybir.ActivationFunctionType.Exp
DR = mybir.MatmulPerfMode.DoubleRow
NP8 = ml_dtypes.float8_e4m3fn


def build_nc(phases=None):
    nc = bacc.Bacc("TRN2", target_bir_lowering=False, debug=False)
    x8 = nc.dram_tensor("x8", [E, 2, T], F8, kind="ExternalInput").ap()
    wqh = nc.dram_tensor("wqh", [E, 2, S], F8, kind="ExternalInput").ap()
    wkh = nc.dram_tensor("wkh", [E, 2, S], F8, kind="ExternalInput").ap()
    wvh = nc.dram_tensor("wvh", [E, 2, S], F8, kind="ExternalInput").ap()
    wql = nc.dram_tensor("wql", [E, S], F8, kind="ExternalInput").ap()
    wkl = nc.dram_tensor("wkl", [E, S], F8, kind="ExternalInput").ap()
    wvl = nc.dram_tensor("wvl", [E, S], F8, kind="ExternalInput").ap()
    wo = nc.dram_tensor("wo", [S, E], F16, kind="ExternalInput").ap()
    bq = nc.dram_tensor("bq", [S, 1], F32, kind="ExternalInput").ap()
    bk = nc.dram_tensor("bk", [S, 1], F32, kind="ExternalInput").ap()
    bv = nc.dram_tensor("bv", [1, S], F32, kind="ExternalInput").ap()
    masks = nc.dram_tensor("masks", [4, 128, 1024], F16, kind="ExternalInput").ap()
    out = nc.dram_tensor("out", [T, E], F16, kind="ExternalOutput").ap()

    with tile.TileContext(nc) as tc:
        _emit(nc, tc, x8, (wqh, wkh, wvh), (wql, wkl, wvl), wo, bq, bk, bv,
              masks, out, phases=phases)
    nc.compile()
    return nc


def _emit(nc, tc, x8, whs, wls, wo, bq, bk, bv, masks, out, phases=None):
    wqh, wkh, wvh = whs
    wql, wkl, wvl = wls
    ctx = ExitStack()
    consts = ctx.enter_context(tc.tile_pool(name="consts", bufs=1))
    mm_ps = ctx.enter_context(tc.tile_pool(name="mm_ps", bufs=2, space="PSUM"))
    st_ps = ctx.enter_context(tc.tile_pool(name="st_ps", bufs=2, space="PSUM"))
    ot_ps = ctx.enter_context(tc.tile_pool(name="ot_ps", bufs=2, space="PSUM"))
    pt_pool = ctx.enter_context(tc.tile_pool(name="pt", bufs=6))
    p8_pool = ctx.enter_context(tc.tile_pool(name="p8", bufs=6))
    sm_pool = ctx.enter_context(tc.tile_pool(name="sm", bufs=4))
    ob_pool = ctx.enter_context(tc.tile_pool(name="ob", bufs=4))
    dr_pool = ctx.enter_context(tc.tile_pool(name="dr", bufs=8, space="DRAM"))

    # --- constant tiles ---
    x_sb = consts.tile([128, KE, 2, T], F8)       # (hi, lo) of x^T
    wqh_sb = consts.tile([128, KE, 2, S], F8)     # duplicated hi weights
    wkh_sb = consts.tile([128, KE, 2, S], F8)
    wvh_sb = consts.tile([128, KE, 2, S], F8)
    wql_sb = consts.tile([128, KE, S], F8)        # lo weights
    wkl_sb = consts.tile([128, KE, S], F8)
    wvl_sb = consts.tile([128, KE, S], F8)
    wo_sb = consts.tile([128, S // 128, E], F16)
    bq_sb = consts.tile([128, 2], F32)
    bk_sb = consts.tile([128, 2], F32)
    bv_bc = consts.tile([128, S], F32)
    mask_sb = consts.tile([128, 4, 1024], F16)
    qt_sb = consts.tile([128, 2, T], F16)
    kt_sb = consts.tile([128, 2, T], F16)
    q8_sb = consts.tile([128, 2, 2, T], F8)       # fp8 q, slot 1 zero-padded
    k8_sb = consts.tile([128, 2, 2, T], F8)       # fp8 k, slot 1 zero-padded
    v_sb = consts.tile([128, NKT, HPC, D + 1], F16)
    v8h_sb = consts.tile([128, NKT, HPC, VP], F8)
    v8l_sb = consts.tile([128, NKT, HPC, VP], F8)
    attnT_sb = consts.tile([128, 2, T], F16)

    # batched constant loads, ordered so the first QK group can start after
    # wq + the first x query-block instead of the full x tensor
    xr = x8.rearrange("(ke p) two t -> p ke two t", p=128)
    wqhr = wqh.rearrange("(ke p) two s -> p ke two s", p=128)
    nc.sync.dma_start(out=wqh_sb[:, 0:4], in_=wqhr[:, 0:4])
    for i in range(2):
        nc.sync.dma_start(out=x_sb[:, 0:4, i, 0:512], in_=xr[:, 0:4, i, 0:512])
    nc.sync.dma_start(out=wqh_sb[:, 4:8], in_=wqhr[:, 4:8])
    for i in range(2):
        nc.sync.dma_start(out=x_sb[:, 4:8, i, 0:512], in_=xr[:, 4:8, i, 0:512])
    # order: everything attn(0,0) needs (q, k, masks, v) before the
    # remaining x query blocks; the start is DMA-serial so order = latency
    nc.sync.dma_start(out=wql_sb, in_=wql.rearrange("(ke p) s -> p ke s", p=128))
    nc.sync.dma_start(out=bq_sb, in_=bq.rearrange("(a p) one -> p (a one)", p=128))
    nc.sync.dma_start(out=wkh_sb, in_=wkh.rearrange("(ke p) two s -> p ke two s", p=128))
    nc.sync.dma_start(out=wkl_sb, in_=wkl.rearrange("(ke p) s -> p ke s", p=128))
    nc.sync.dma_start(out=bk_sb, in_=bk.rearrange("(a p) one -> p (a one)", p=128))
    nc.sync.dma_start(out=mask_sb, in_=masks.rearrange("r p n -> p r n"))
    nc.sync.dma_start(out=wvh_sb, in_=wvh.rearrange("(ke p) two s -> p ke two s", p=128))
    nc.sync.dma_start(out=wvl_sb, in_=wvl.rearrange("(ke p) s -> p ke s", p=128))
    nc.sync.dma_start(out=bv_bc, in_=bv.to_broadcast((128, S)))
    for qb in range(1, NQB):
        qs = slice(qb * 512, (qb + 1) * 512)
        for i in range(2):
            nc.sync.dma_start(out=x_sb[:, :, i, qs], in_=xr[:, :, i, qs])
    nc.sync.dma_start(out=wo_sb, in_=wo.rearrange("(a p) n -> p a n", p=128))
    nc.vector.memset(v_sb[:, :, :, D : D + 1], SC)
    # warm the exp activation table during the initial DMA wait
    warm = consts.tile([1, 1], F32)
    nc.vector.memset(warm, 0.0)
    nc.scalar.activation(warm, warm, EXP)

    def emit_presets():
        # zero pads/slots read by the DoubleRow matmuls; not needed until the
        # first lower-tile block, so these sit after attn(0,0) in Pool's
        # queue (they are disjoint from the slot-0/value writes)
        nc.gpsimd.memset(v8h_sb[:, :, :, D : D + 1], SC)
        nc.gpsimd.memset(v8h_sb[:, :, :, D + 1 : VP], 0.0)
        nc.gpsimd.memset(v8l_sb[:, :, :, D : VP], 0.0)
        nc.gpsimd.memset(q8_sb[:, :, 1, :], 0.0)
        nc.gpsimd.memset(k8_sb[:, :, 1, :], 0.0)

    # --- V = x @ wv + bv (fp8 hi-lo DoubleRow), plus fp8 hi/lo splits ---
    def emit_v(rt):
        rsl = slice(rt * 128, (rt + 1) * 128)
        ps = mm_ps.tile([128, 512], F32, tag="mm", name=f"vps{rt}")
        for ke in range(KE):
            nc.tensor.matmul(
                ps[:, 0:S],
                lhsT=x_sb[:, ke, :, rsl],
                rhs=wvh_sb[:, ke],
                start=(ke == 0),
                stop=False,
                perf_mode=DR,
            )
        for a in range(KE // 2):
            nc.tensor.matmul(
                ps[:, 0:S],
                lhsT=x_sb[:, 2 * a : 2 * a + 2, 0, rsl],
                rhs=wvl_sb[:, 2 * a : 2 * a + 2, :],
                start=False,
                stop=(a == KE // 2 - 1),
                perf_mode=DR,
            )
        nc.vector.tensor_add(
            v_sb[:, rt, :, 0:D],
            ps[:, 0:S].rearrange("p (h d) -> p h d", h=HPC),
            bv_bc.rearrange("p (h d) -> p h d", h=HPC),
        )
        nc.gpsimd.tensor_copy(v8h_sb[:, rt, :, 0:D], v_sb[:, rt, :, 0:D])
        nc.gpsimd.tensor_sub(
            v8l_sb[:, rt, :, 0:D], v_sb[:, rt, :, 0:D], v8h_sb[:, rt, :, 0:D]
        )

    # --- QT/KT = (x @ w + b).T for one pair of heads (128 cols) ---
    def emit_qk_part(p, qb, which):
        qs = slice(qb * 512, (qb + 1) * 512)
        psl = slice(p * 128, (p + 1) * 128)
        wh_sb, wl_sb, b_sb, dst, nm = {
            "q": (wqh_sb, wql_sb, bq_sb, qt_sb, "q"),
            "k": (wkh_sb, wkl_sb, bk_sb, kt_sb, "k"),
        }[which]
        ps = mm_ps.tile([128, 512], F32, tag="mm", name=f"{nm}ps{p}_{qb}")
        for ke in range(KE):
            nc.tensor.matmul(
                ps,
                lhsT=wh_sb[:, ke, :, psl],
                rhs=x_sb[:, ke, :, qs],
                start=(ke == 0),
                stop=False,
                perf_mode=DR,
            )
        for a in range(KE // 2):
            nc.tensor.matmul(
                ps,
                lhsT=wl_sb[:, 2 * a : 2 * a + 2, psl],
                rhs=x_sb[:, 2 * a : 2 * a + 2, 0, qs],
                start=False,
                stop=(a == KE // 2 - 1),
                perf_mode=DR,
            )
        nc.vector.tensor_scalar_add(dst[:, p, qs], ps, b_sb[:, p : p + 1])
        # fp8 copies for the zero-padded DoubleRow lower-tile scores:
        # q8 needed for query blocks >=1, k8 for key blocks <=2
        if which == "q" and qb > 0:
            nc.vector.tensor_copy(q8_sb[:, p, 0, qs], dst[:, p, qs])
        if which == "k" and qb < 3:
            nc.gpsimd.tensor_copy(k8_sb[:, p, 0, qs], dst[:, p, qs])

    # --- attention for pair p (heads 2p, 2p+1), query block qb ---
    def emit_attn(p, qb, fill, last=False):
        fill.tick()
        qs = slice(qb * 512, (qb + 1) * 512)
        ots = [
            ot_ps.tile([VP, 512], F32, tag="ot", name=f"ot{p}_{qb}_{i}")
            for i in range(2)
        ]

        def do_st8(kt):
            # fp8 zero-padded DoubleRow score tile (strictly-lower keys)
            st = st_ps.tile([128, 1024], F32, tag="st", name=f"st{p}_{qb}_{kt}")
            for hh in range(2):
                hs = slice(hh * 64, (hh + 1) * 64)
                nc.tensor.matmul(
                    st[:, hh * 512 : (hh + 1) * 512],
                    lhsT=k8_sb[hs, p, :, kt * 128 : (kt + 1) * 128],
                    rhs=q8_sb[hs, p, :, qs],
                    start=True,
                    stop=True,
                    perf_mode=DR,
                )
            return st

        def do_st16(kt, off):
            st = st_ps.tile([128, 1024], F32, tag="st", name=f"st{p}_{qb}_{kt}")
            for hh in range(2):
                hs = slice(hh * 64, (hh + 1) * 64)
                nc.tensor.matmul(
                    st[:, hh * 512 + off : (hh + 1) * 512],
                    lhsT=kt_sb[hs, p, kt * 128 : (kt + 1) * 128],
                    rhs=qt_sb[hs, p, qb * 512 + off : (qb + 1) * 512],
                    start=True,
                    stop=True,
                )
            return st

        # lower key tiles: fp8 probs, DoubleRow AV over (hi, lo) V splits
        for a in range(2 * qb):
            st0 = do_st8(2 * a)
            fill.pop(diag=a >= qb)
            st1 = do_st8(2 * a + 1)
            pt8 = p8_pool.tile([128, 2, 1024], F8, tag="p8", name=f"p8_{p}_{qb}_{a}")
            nc.scalar.activation(pt8[:, 0, :], st0, EXP, scale=ES)
            nc.scalar.activation(pt8[:, 1, :], st1, EXP, scale=ES)
            for hh in range(2):
                rhs8 = pt8[:, :, hh * 512 : (hh + 1) * 512]
                for v8 in (v8h_sb, v8l_sb):
                    nc.tensor.matmul(
                        ots[hh],
                        lhsT=v8[:, 2 * a : 2 * a + 2, 2 * p + hh, :],
                        rhs=rhs8,
                        start=(a == 0 and v8 is v8h_sb),
                        stop=False,
                        perf_mode=DR,
                        skip_group_check=(a != 0 or v8 is not v8h_sb),
                    )
            fill.pop(diag=a >= qb)

        # diagonal tiles: fp16 probs, multiplicative causal mask, trimmed.
        # A full-width matmul opens the accumulation group (qb=0 only) and
        # another closes it.
        if qb == 0:
            diag_order = [(0, 0), (2, 256), (3, 384), (1, 0)]
        else:
            diag_order = [(1, 128), (2, 256), (3, 384), (0, 0)]
        for i, (r, off) in enumerate(diag_order):
            kt = 4 * qb + r
            st = do_st16(kt, off)
            fill.pop(diag=True)
            pt = pt_pool.tile([128, 1024], F16, tag="pt", name=f"pt{p}_{qb}_{kt}")
            if off:
                stv = st.rearrange("p (a n) -> p a n", a=2)
                ptv = pt.rearrange("p (a n) -> p a n", a=2)
                mkv = mask_sb[:, r, :].rearrange("p (a n) -> p a n", a=2)
                nc.scalar.activation(ptv[:, :, off:512], stv[:, :, off:512], EXP, scale=ES)
                nc.vector.tensor_mul(
                    ptv[:, :, off:512], ptv[:, :, off:512], mkv[:, :, off:512]
                )
            else:
                nc.scalar.activation(pt, st, EXP, scale=ES)
                nc.vector.tensor_mul(pt, pt, mask_sb[:, r, :])
            start = qb == 0 and i == 0
            stop = i == len(diag_order) - 1
            for hh in range(2):
                nc.tensor.matmul(
                    ots[hh][0 : D + 1, off:512],
                    lhsT=v_sb[:, kt, 2 * p + hh, :],
                    rhs=pt[:, hh * 512 + off : (hh + 1) * 512],
                    start=start,
                    stop=stop,
                    skip_group_check=not start,
                )
            fill.pop(diag=True)

        # normalization: copy both heads' OT out of PSUM, fp32 reciprocal of
        # the rowsum rows (which carry the 1/32 value-scale: the rowsum
        # column of V is 32.0), one DRAM-bounce broadcast, two gpsimd muls.
        # The last-emitted block's chain is the kernel tail: parallelize its
        # copies/muls across engines (ScalarE and DVE are idle by then).
        oc = sm_pool.tile([D + 1, 2, 512], F32, tag="oc", name=f"oc{p}_{qb}")
        nc.vector.tensor_copy(oc[:, 0, :], ots[0][0 : D + 1, :])
        if last:
            nc.scalar.copy(oc[:, 1, :], ots[1][0 : D + 1, :])
        else:
            nc.vector.tensor_copy(oc[:, 1, :], ots[1][0 : D + 1, :])
        if last:
            # tail-latency variant: bounce the RAW rowsums (skipping the
            # repartition DMA) and take the reciprocal after the broadcast
            # on the by-then-idle DVE - one less serial DMA in the final
            # normalize->Wo chain
            rd = dr_pool.tile([1, 2, 512], F32, tag="rd", name=f"rd{p}_{qb}")
            nc.sync.dma_start(out=rd, in_=oc[D : D + 1, :, :])
            rbr = sm_pool.tile([D, 2, 512], F32, tag="rbr", name=f"rbr{p}_{qb}",
                               bufs=1)
            nc.sync.dma_start(out=rbr, in_=rd.to_broadcast((D, 2, 512)))
            rbc = sm_pool.tile([D, 2, 512], F32, tag="rbc", name=f"rbc{p}_{qb}")
            nc.vector.reciprocal(rbc, rbr)
        else:
            rsq = sm_pool.tile([D, 16], F32, tag="rsq", name=f"rsq{p}_{qb}")
            nc.sync.dma_start(out=rsq, in_=oc[D : D + 1, :, :])
            rr = sm_pool.tile([D, 16], F32, tag="rr", name=f"rr{p}_{qb}")
            nc.vector.reciprocal(rr, rsq)
            rd = dr_pool.tile([1, 2, 512], F32, tag="rd", name=f"rd{p}_{qb}")
            nc.sync.dma_start(out=rd, in_=rr)
            rbc = sm_pool.tile([D, 2, 512], F32, tag="rbc", name=f"rbc{p}_{qb}")
            nc.sync.dma_start(out=rbc, in_=rd.to_broadcast((D, 2, 512)))
        stg = sm_pool.tile([D, 512], F16, tag="stg", name=f"stg{p}_{qb}")
        if last:
            nc.vector.tensor_mul(attnT_sb[0:D, p, qs], oc[0:D, 0, :], rbc[:, 0, :])
        else:
            nc.gpsimd.tensor_mul(attnT_sb[0:D, p, qs], oc[0:D, 0, :], rbc[:, 0, :])
        nc.gpsimd.tensor_mul(stg, oc[0:D, 1, :], rbc[:, 1, :])
        nc.sync.dma_start(out=attnT_sb[D:128, p, qs], in_=stg)

    # --- output projection: out tile = attnT.T @ wo ---
    def emit_wo_part(qt, tail=False):
        o_sb = ob_pool.tile([128, 1024], F16, tag="ob", name=f"ob{qt}")
        for nt in range(2):
            # in the tail, attention is done: borrow the idle score-PSUM pool
            # for every other tile to double the accumulate/copy rotation
            pool = st_ps if tail and nt == 1 else mm_ps
            tag = "st" if pool is st_ps else "mm"
            ps = pool.tile([128, 512], F32, tag=tag, name=f"ops{qt}_{nt}")
            for p in range(2):
                nc.tensor.matmul(
                    ps,
                    lhsT=attnT_sb[:, p, qt * 128 : (qt + 1) * 128],
                    rhs=wo_sb[:, p, nt * 512 : (nt + 1) * 512],
                    start=(p == 0),
                    stop=(p == 1),
                )
            # tail tiles split copies DVE/ScalarE (both idle at the end)
            if tail and nt == 1:
                nc.scalar.copy(o_sb[:, nt * 512 : (nt + 1) * 512], ps)
            else:
                nc.vector.tensor_copy(o_sb[:, nt * 512 : (nt + 1) * 512], ps)
        nc.sync.dma_start(out=out[qt * 128 : (qt + 1) * 128, :], in_=o_sb)

    class Fill:
        """FIFO of independent emission units; deps are enforced by the tile
        framework's semaphores, so order only affects performance. "late"
        units (Wo tiles, whose attnT producer chain is freshly emitted) pop
        only once mature - in the diagonal phase one block later, or anywhere
        two blocks later - so they neither head-of-line block the PE queue
        nor burst back-to-back (which would starve ScalarE via the mm-pool/
        DVE copy rotation)."""

        def __init__(self):
            self.q = []
            self.done = set()
            self.block = 0
            self.late_budget = 2

        def tick(self):
            self.block += 1
            self.late_budget = 2

        def add(self, key, fn, late=False):
            if key not in self.done:
                self.q.append((key, fn, self.block if late else -2))

        def pop(self, n=1, diag=False):
            for _ in range(n):
                idx = None
                fresh = False
                for i, (key, fn, birth) in enumerate(self.q):
                    if key in self.done:
                        continue
                    age = self.block - birth
                    if age < 1:
                        continue
                    if age == 1 and not (diag and self.late_budget > 0):
                        continue
                    idx = i
                    fresh = age == 1
                    break
                if idx is None:
                    return
                if fresh:
                    self.late_budget -= 1
                key, fn, _ = self.q.pop(idx)
                self.done.add(key)
                fn()

        def ensure(self, key, fn, late=False):
            if key not in self.done:
                self.done.add(key)
                fn()

        def flush(self):
            for key, fn, _ in list(self.q):
                if key not in self.done:
                    self.done.add(key)
                    fn()
            self.q = []

    fill = Fill()

    def qk_unit(p, qb, which):
        return (("qk", p, qb, which), lambda: emit_qk_part(p, qb, which))

    def v_unit(rt):
        return (("v", rt), lambda: emit_v(rt))

    tail_mode = [False]

    def wo_unit(qt):
        return (("wo", qt), lambda: emit_wo_part(qt, tail=tail_mode[0]))

    if phases is not None:
        # bisection mode: simple phase ordering, no fillers
        emit_presets()
        nofill = Fill()
        if phases is not None and "qk" in phases or phases is None:
            pass
        if "qk" in phases:
            for qb in range(NQB):
                for w in "qk":
                    emit_qk_part(0, qb, w)
                for w in "qk":
                    emit_qk_part(1, qb, w)
        if "v" in phases:
            for rt in range(NKT):
                emit_v(rt)
        if "attn" in phases:
            for qb in range(NQB):
                emit_attn(0, qb, nofill)
            for qb in range(NQB):
                emit_attn(1, qb, nofill)
        if "wo" in phases:
            for qt in range(NKT):
                emit_wo_part(qt)
    else:
        # pair-0 phase: V tiles, later pair-0 QK parts and early pair-1 QK
        # parts ride as fillers inside the exp-paced attention blocks
        for qb in range(NQB):
            fill.ensure(*qk_unit(0, qb, "q"))
            fill.ensure(*qk_unit(0, qb, "k"))
            for rt in range(4 * qb, 4 * qb + 4):
                fill.ensure(*v_unit(rt))
            if qb < NQB - 1:
                for rt in range(4 * qb + 4, 4 * qb + 8):
                    fill.add(*v_unit(rt))
                fill.add(*qk_unit(0, qb + 1, "q"))
                fill.add(*qk_unit(0, qb + 1, "k"))
            if qb == NQB - 1:
                # qk(1,0) must precede ALL pair-1 blocks: its k8 columns
                # 0:512 feed every pair-1 lower tile
                fill.add(*qk_unit(1, 0, "k"))
                fill.add(*qk_unit(1, 1, "q"))
                fill.add(*qk_unit(1, 1, "k"))
            emit_attn(0, qb, fill)
            if qb == 0:
                emit_presets()
        # pair-1 phase ordered (1, 2, 0, 3): each block's fillers are later
        # blocks' QK parts plus the Wo tiles unlocked so far ("late": popped
        # only once their attnT normalization has had time to land, rate-
        # capped to avoid bursts that starve ScalarE). Ending on qb=3 (the
        # largest block) absorbs the most Wo work; only its own norm chain
        # and Wo tiles (12-15) remain as the tail. Wo units are emitted
        # strictly after their producing normalization (emission order
        # defines dependency direction in the tile framework).
        fill.flush()
        fill.ensure(*qk_unit(1, 1, "q"))
        fill.ensure(*qk_unit(1, 1, "k"))
        fill.add(*qk_unit(1, 2, "q"))
        fill.add(*qk_unit(1, 2, "k"))
        emit_attn(1, 1, fill)
        fill.ensure(*qk_unit(1, 2, "q"))
        fill.ensure(*qk_unit(1, 2, "k"))
        fill.add(*qk_unit(1, 0, "q"))
        fill.add(*qk_unit(1, 0, "k"))
        fill.add(*qk_unit(1, 3, "q"))
        fill.add(*qk_unit(1, 3, "k"))
        for qt in range(4, 8):
            fill.add(*wo_unit(qt), late=True)
        emit_attn(1, 2, fill)
        fill.ensure(*qk_unit(1, 0, "q"))
        fill.ensure(*qk_unit(1, 0, "k"))
        for qt in range(8, 12):
            fill.add(*wo_unit(qt), late=True)
        emit_attn(1, 0, fill)
        fill.ensure(*qk_unit(1, 3, "q"))
        fill.ensure(*qk_unit(1, 3, "k"))
        for qt in range(0, 4):
            fill.add(*wo_unit(qt), late=True)
        emit_attn(1, 3, fill, last=True)
        tail_mode[0] = True
        fill.flush()
        for qt in range(12, 16):
            emit_wo_part(qt, tail=True)
    ctx.close()


def make_masks():
    i = np.arange(128)[:, None]
    j = np.arange(512)[None, :]
    m = np.stack([(i + 128 * r <= j) for r in range(4)], axis=0).astype(np.float16)
    return np.concatenate([m, m], axis=2)  # duplicated per head pair


def _hilo(a):
    hi = a.astype(NP8)
    lo = (a - hi.astype(np.float32)).astype(NP8)
    return hi, lo


def make_in_maps(x, Wq, bq, Wk, bk, Wv, bv, Wo):
    masks = make_masks()
    in_maps = []
    x8b = []
    for b in range(2):
        xT = np.ascontiguousarray(x[b].T.astype(np.float32))
        xh, xl = _hilo(xT)
        x8b.append(np.ascontiguousarray(np.stack([xh, xl], axis=1)))
    wsplits = {}
    for nm, W in (("q", Wq), ("k", Wk), ("v", Wv)):
        for hg in range(4):
            sl = slice(hg * S, (hg + 1) * S)
            wh, wl = _hilo(np.asarray(W[:, sl], np.float32) * SC)
            whd = np.ascontiguousarray(np.stack([wh, wh], axis=1))
            wsplits[(nm, hg)] = (whd, np.ascontiguousarray(wl))
    for c in range(8):
        b, hg = divmod(c, 4)
        sl = slice(hg * S, (hg + 1) * S)
        wqh_, wql_ = wsplits[("q", hg)]
        wkh_, wkl_ = wsplits[("k", hg)]
        wvh_, wvl_ = wsplits[("v", hg)]
        in_maps.append(
            {
                "x8": x8b[b],
                "wqh": wqh_, "wql": wql_,
                "wkh": wkh_, "wkl": wkl_,
                "wvh": wvh_, "wvl": wvl_,
                "wo": np.ascontiguousarray(Wo[sl, :].astype(np.float16)),
                "bq": np.ascontiguousarray((SC * bq[sl]).astype(np.float32).reshape(S, 1)),
                "bk": np.ascontiguousarray((SC * bk[sl]).astype(np.float32).reshape(S, 1)),
                "bv": np.ascontiguousarray((SC * bv[sl]).astype(np.float32).reshape(1, S)),
                "masks": masks,
            }
        )
    return in_maps


_NC_CACHE = None


def _get_nc():
    global _NC_CACHE
    if _NC_CACHE is None:
        _NC_CACHE = build_nc()
    return _NC_CACHE


def _run(x, Wq, bq, Wk, bk, Wv, bv, Wo, bo, trace=False, **spmd_kwargs):
    nc = _get_nc()
    in_maps = make_in_maps(
        np.asarray(x), np.asarray(Wq), np.asarray(bq), np.asarray(Wk),
        np.asarray(bk), np.asarray(Wv), np.asarray(bv), np.asarray(Wo),
    )
    res = run_bass_kernel_spmd(
        nc, in_maps, core_ids=list(range(8)), trace=trace, **spmd_kwargs
    )
    out = np.zeros((2, T, E), dtype=np.float32)
    for c in range(8):
        out[c // 4] += res.results[c]["out"]
    out += np.asarray(bo, dtype=np.float32)[None, None, :]
    return out, res


def kernel(x, Wq, bq, Wk, bk, Wv, bv, Wo, bo):
    out, _ = _run(x, Wq, bq, Wk, bk, Wv, bv, Wo, bo)
    return out


# revision 57
# speedup vs baseline: 1.1180x; 1.0048x over previous
"""Causal self-attention (B=2, T=2048, E=1024, H=16, D=64) on 8 TRN2 NeuronCores.

Sharding: core = (batch b, head-group hg): 2 batches x 4 head-groups of 4 heads.
Each core computes QKV projections for its 4 heads (256 columns), causal
attention, and the output projection against its 256 rows of Wo, producing a
partial [2048, 1024] output. Host sums the 4 head-group partials per batch
(the tensor-parallel all-reduce) and adds bo.

Precision/performance structure (matmul cost goes by output columns; fp8e4
DoubleRow pairs two K-tiles per instruction at half cost):
  - QKV projections in fp8 hi+lo split precision: W' = 32*W and x decompose
    as hi8 + lo8 host-side; (whi+wlo)@x_hi + whi@x_lo runs as DoubleRow pairs
    for 0.75x the fp16 cost at ~fp16 accuracy. The 32x weight scale keeps lo
    residuals above e4m3's subnormal floor; it is folded back via the exp
    scale and V's 32.0 rowsum column.
  - Scores per head pair: diagonal-block tiles fp16; strictly-lower tiles use
    fp8 q/k via a zero-padded DoubleRow (half cost). Lower-tile probs
    quantize to fp8 and feed DoubleRow AV against V's hi8+lo8 split (half
    cost); diagonal tiles stay fp16 (multiplicative causal mask), so every
    query's nearest <=512 keys are full precision - far-key quantization
    noise averages out in the softmax.
  - Emission interleaves independent matmul "filler" units (V tiles, next
    QK parts, output-projection tiles) into the attention instruction stream:
    the PE queue is in-order, so without fillers it head-of-line blocks on
    ScalarE exp between score and AV instructions.
"""
from contextlib import ExitStack

import numpy as np
import ml_dtypes

import concourse.bass as bass  # noqa: F401
import concourse.mybir as mybir
import concourse.tile as tile
from concourse import bacc
from concourse.bass_utils import run_bass_kernel_spmd

T = 2048
E = 1024
HPC = 4          # heads per core
D = 64
S = HPC * D      # 256: per-core head-column slice
KE = E // 128    # 8 contraction tiles for the projections
NKT = T // 128   # 16 key row tiles
NQB = T // 512   # 4 query column blocks
VP = 96          # padded V columns for DoubleRow AV (64 vals + rowsum + pad)
SC = 32.0        # weight scale: keeps fp8 lo residuals out of subnormals
ES = 0.125 / (SC * SC)  # exp scale with the q/k scale folded in
F8 = mybir.dt.float8e4
F16 = mybir.dt.float16
F32 = mybir.dt.float32
EXP = mybir.ActivationFunctionType.Exp
DR = mybir.MatmulPerfMode.DoubleRow
NP8 = ml_dtypes.float8_e4m3fn


def build_nc(phases=None):
    nc = bacc.Bacc("TRN2", target_bir_lowering=False, debug=False)
    x8 = nc.dram_tensor("x8", [E, 2, T], F8, kind="ExternalInput").ap()
    wqh = nc.dram_tensor("wqh", [E, 2, S], F8, kind="ExternalInput").ap()
    wkh = nc.dram_tensor("wkh", [E, 2, S], F8, kind="ExternalInput").ap()
    wvh = nc.dram_tensor("wvh", [E, 2, S], F8, kind="ExternalInput").ap()
    wql = nc.dram_tensor("wql", [E, S], F8, kind="ExternalInput").ap()
    wkl = nc.dram_tensor("wkl", [E, S], F8, kind="ExternalInput").ap()
    wvl = nc.dram_tensor("wvl", [E, S], F8, kind="ExternalInput").ap()
    wo = nc.dram_tensor("wo", [S, E], F16, kind="ExternalInput").ap()
    bq = nc.dram_tensor("bq", [S, 1], F32, kind="ExternalInput").ap()
    bk = nc.dram_tensor("bk", [S, 1], F32, kind="ExternalInput").ap()
    bv = nc.dram_tensor("bv", [1, S], F32, kind="ExternalInput").ap()
    masks = nc.dram_tensor("masks", [4, 128, 1024], F16, kind="ExternalInput").ap()
    out = nc.dram_tensor("out", [T, E], F16, kind="ExternalOutput").ap()

    with tile.TileContext(nc) as tc:
        _emit(nc, tc, x8, (wqh, wkh, wvh), (wql, wkl, wvl), wo, bq, bk, bv,
              masks, out, phases=phases)
    nc.compile()
    return nc


def _emit(nc, tc, x8, whs, wls, wo, bq, bk, bv, masks, out, phases=None):
    wqh, wkh, wvh = whs
    wql, wkl, wvl = wls
    ctx = ExitStack()
    consts = ctx.enter_context(tc.tile_pool(name="consts", bufs=1))
    mm_ps = ctx.enter_context(tc.tile_pool(name="mm_ps", bufs=2, space="PSUM"))
    st_ps = ctx.enter_context(tc.tile_pool(name="st_ps", bufs=2, space="PSUM"))
    ot_ps = ctx.enter_context(tc.tile_pool(name="ot_ps", bufs=2, space="PSUM"))
    pt_pool = ctx.enter_context(tc.tile_pool(name="pt", bufs=6))
    p8_pool = ctx.enter_context(tc.tile_pool(name="p8", bufs=6))
    sm_pool = ctx.enter_context(tc.tile_pool(name="sm", bufs=4))
    ob_pool = ctx.enter_context(tc.tile_pool(name="ob", bufs=4))
    dr_pool = ctx.enter_context(tc.tile_pool(name="dr", bufs=8, space="DRAM"))

    # --- constant tiles ---
    x_sb = consts.tile([128, KE, 2, T], F8)       # (hi, lo) of x^T
    wqh_sb = consts.tile([128, KE, 2, S], F8)     # duplicated hi weights
    wkh_sb = consts.tile([128, KE, 2, S], F8)
    wvh_sb = consts.tile([128, KE, 2, S], F8)
    wql_sb = consts.tile([128, KE, S], F8)        # lo weights
    wkl_sb = consts.tile([128, KE, S], F8)
    wvl_sb = consts.tile([128, KE, S], F8)
    wo_sb = consts.tile([128, S // 128, E], F16)
    bq_sb = consts.tile([128, 2], F32)
    bk_sb = consts.tile([128, 2], F32)
    bv_bc = consts.tile([128, S], F32)
    mask_sb = consts.tile([128, 4, 1024], F16)
    qt_sb = consts.tile([128, 2, T], F16)
    kt_sb = consts.tile([128, 2, T], F16)
    q8_sb = consts.tile([128, 2, 2, T], F8)       # fp8 q, slot 1 zero-padded
    k8_sb = consts.tile([128, 2, 2, T], F8)       # fp8 k, slot 1 zero-padded
    v_sb = consts.tile([128, NKT, HPC, D + 1], F16)
    v8h_sb = consts.tile([128, NKT, HPC, VP], F8)
    v8l_sb = consts.tile([128, NKT, HPC, VP], F8)
    attnT_sb = consts.tile([128, 2, T], F16)

    # batched constant loads, ordered so the first QK group can start after
    # wq + the first x query-block instead of the full x tensor
    xr = x8.rearrange("(ke p) two t -> p ke two t", p=128)
    wqhr = wqh.rearrange("(ke p) two s -> p ke two s", p=128)
    nc.sync.dma_start(out=wqh_sb[:, 0:4], in_=wqhr[:, 0:4])
    for i in range(2):
        nc.sync.dma_start(out=x_sb[:, 0:4, i, 0:512], in_=xr[:, 0:4, i, 0:512])
    nc.sync.dma_start(out=wqh_sb[:, 4:8], in_=wqhr[:, 4:8])
    for i in range(2):
        nc.sync.dma_start(out=x_sb[:, 4:8, i, 0:512], in_=xr[:, 4:8, i, 0:512])
    # order: everything attn(0,0) needs (q, k, masks, v) before the
    # remaining x query blocks; the start is DMA-serial so order = latency
    nc.sync.dma_start(out=wql_sb, in_=wql.rearrange("(ke p) s -> p ke s", p=128))
    nc.sync.dma_start(out=bq_sb, in_=bq.rearrange("(a p) one -> p (a one)", p=128))
    nc.sync.dma_start(out=wkh_sb, in_=wkh.rearrange("(ke p) two s -> p ke two s", p=128))
    nc.sync.dma_start(out=wkl_sb, in_=wkl.rearrange("(ke p) s -> p ke s", p=128))
    nc.sync.dma_start(out=bk_sb, in_=bk.rearrange("(a p) one -> p (a one)", p=128))
    # masks split: only r=0 gates attn(0,0)'s first masked tile; r1-3 follow
    # the V weights so the first AV chain isn't serialized behind the full 1MB
    mr = masks.rearrange("r p n -> p r n")
    nc.sync.dma_start(out=mask_sb[:, 0:1], in_=mr[:, 0:1])
    nc.sync.dma_start(out=wvh_sb, in_=wvh.rearrange("(ke p) two s -> p ke two s", p=128))
    nc.sync.dma_start(out=wvl_sb, in_=wvl.rearrange("(ke p) s -> p ke s", p=128))
    nc.sync.dma_start(out=bv_bc, in_=bv.to_broadcast((128, S)))
    nc.sync.dma_start(out=mask_sb[:, 1:4], in_=mr[:, 1:4])
    for qb in range(1, NQB):
        qs = slice(qb * 512, (qb + 1) * 512)
        for i in range(2):
            nc.sync.dma_start(out=x_sb[:, :, i, qs], in_=xr[:, :, i, qs])
    nc.sync.dma_start(out=wo_sb, in_=wo.rearrange("(a p) n -> p a n", p=128))
    nc.vector.memset(v_sb[:, :, :, D : D + 1], SC)
    # warm the exp activation table during the initial DMA wait
    warm = consts.tile([1, 1], F32)
    nc.vector.memset(warm, 0.0)
    nc.scalar.activation(warm, warm, EXP)

    def emit_presets():
        # zero pads/slots read by the DoubleRow matmuls; not needed until the
        # first lower-tile block, so these sit after attn(0,0) in Pool's
        # queue (they are disjoint from the slot-0/value writes)
        nc.gpsimd.memset(v8h_sb[:, :, :, D : D + 1], SC)
        nc.gpsimd.memset(v8h_sb[:, :, :, D + 1 : VP], 0.0)
        nc.gpsimd.memset(v8l_sb[:, :, :, D : VP], 0.0)
        nc.gpsimd.memset(q8_sb[:, :, 1, :], 0.0)
        nc.gpsimd.memset(k8_sb[:, :, 1, :], 0.0)

    # --- V = x @ wv + bv (fp8 hi-lo DoubleRow), plus fp8 hi/lo splits ---
    def emit_v(rt):
        rsl = slice(rt * 128, (rt + 1) * 128)
        ps = mm_ps.tile([128, 512], F32, tag="mm", name=f"vps{rt}")
        for ke in range(KE):
            nc.tensor.matmul(
                ps[:, 0:S],
                lhsT=x_sb[:, ke, :, rsl],
                rhs=wvh_sb[:, ke],
                start=(ke == 0),
                stop=False,
                perf_mode=DR,
            )
        for a in range(KE // 2):
            nc.tensor.matmul(
                ps[:, 0:S],
                lhsT=x_sb[:, 2 * a : 2 * a + 2, 0, rsl],
                rhs=wvl_sb[:, 2 * a : 2 * a + 2, :],
                start=False,
                stop=(a == KE // 2 - 1),
                perf_mode=DR,
            )
        nc.vector.tensor_add(
            v_sb[:, rt, :, 0:D],
            ps[:, 0:S].rearrange("p (h d) -> p h d", h=HPC),
            bv_bc.rearrange("p (h d) -> p h d", h=HPC),
        )
        nc.gpsimd.tensor_copy(v8h_sb[:, rt, :, 0:D], v_sb[:, rt, :, 0:D])
        nc.gpsimd.tensor_sub(
            v8l_sb[:, rt, :, 0:D], v_sb[:, rt, :, 0:D], v8h_sb[:, rt, :, 0:D]
        )

    # --- QT/KT = (x @ w + b).T for one pair of heads (128 cols) ---
    def emit_qk_part(p, qb, which):
        qs = slice(qb * 512, (qb + 1) * 512)
        psl = slice(p * 128, (p + 1) * 128)
        wh_sb, wl_sb, b_sb, dst, nm = {
            "q": (wqh_sb, wql_sb, bq_sb, qt_sb, "q"),
            "k": (wkh_sb, wkl_sb, bk_sb, kt_sb, "k"),
        }[which]
        ps = mm_ps.tile([128, 512], F32, tag="mm", name=f"{nm}ps{p}_{qb}")
        for ke in range(KE):
            nc.tensor.matmul(
                ps,
                lhsT=wh_sb[:, ke, :, psl],
                rhs=x_sb[:, ke, :, qs],
                start=(ke == 0),
                stop=False,
                perf_mode=DR,
            )
        for a in range(KE // 2):
            nc.tensor.matmul(
                ps,
                lhsT=wl_sb[:, 2 * a : 2 * a + 2, psl],
                rhs=x_sb[:, 2 * a : 2 * a + 2, 0, qs],
                start=False,
                stop=(a == KE // 2 - 1),
                perf_mode=DR,
            )
        nc.vector.tensor_scalar_add(dst[:, p, qs], ps, b_sb[:, p : p + 1])
        # fp8 copies for the zero-padded DoubleRow lower-tile scores:
        # q8 needed for query blocks >=1, k8 for key blocks <=2
        if which == "q" and qb > 0:
            nc.vector.tensor_copy(q8_sb[:, p, 0, qs], dst[:, p, qs])
        if which == "k" and qb < 3:
            nc.gpsimd.tensor_copy(k8_sb[:, p, 0, qs], dst[:, p, qs])

    # --- attention for pair p (heads 2p, 2p+1), query block qb ---
    def emit_attn(p, qb, fill, last=False):
        fill.tick()
        qs = slice(qb * 512, (qb + 1) * 512)
        ots = [
            ot_ps.tile([VP, 512], F32, tag="ot", name=f"ot{p}_{qb}_{i}")
            for i in range(2)
        ]

        def do_st8(kt):
            # fp8 zero-padded DoubleRow score tile (strictly-lower keys)
            st = st_ps.tile([128, 1024], F32, tag="st", name=f"st{p}_{qb}_{kt}")
            for hh in range(2):
                hs = slice(hh * 64, (hh + 1) * 64)
                nc.tensor.matmul(
                    st[:, hh * 512 : (hh + 1) * 512],
                    lhsT=k8_sb[hs, p, :, kt * 128 : (kt + 1) * 128],
                    rhs=q8_sb[hs, p, :, qs],
                    start=True,
                    stop=True,
                    perf_mode=DR,
                )
            return st

        def do_st16(kt, off):
            st = st_ps.tile([128, 1024], F32, tag="st", name=f"st{p}_{qb}_{kt}")
            for hh in range(2):
                hs = slice(hh * 64, (hh + 1) * 64)
                nc.tensor.matmul(
                    st[:, hh * 512 + off : (hh + 1) * 512],
                    lhsT=kt_sb[hs, p, kt * 128 : (kt + 1) * 128],
                    rhs=qt_sb[hs, p, qb * 512 + off : (qb + 1) * 512],
                    start=True,
                    stop=True,
                )
            return st

        # lower key tiles: fp8 probs, DoubleRow AV over (hi, lo) V splits
        for a in range(2 * qb):
            st0 = do_st8(2 * a)
            fill.pop(diag=a >= qb)
            st1 = do_st8(2 * a + 1)
            pt8 = p8_pool.tile([128, 2, 1024], F8, tag="p8", name=f"p8_{p}_{qb}_{a}")
            nc.scalar.activation(pt8[:, 0, :], st0, EXP, scale=ES)
            nc.scalar.activation(pt8[:, 1, :], st1, EXP, scale=ES)
            for hh in range(2):
                rhs8 = pt8[:, :, hh * 512 : (hh + 1) * 512]
                for v8 in (v8h_sb, v8l_sb):
                    nc.tensor.matmul(
                        ots[hh],
                        lhsT=v8[:, 2 * a : 2 * a + 2, 2 * p + hh, :],
                        rhs=rhs8,
                        start=(a == 0 and v8 is v8h_sb),
                        stop=False,
                        perf_mode=DR,
                        skip_group_check=(a != 0 or v8 is not v8h_sb),
                    )
            fill.pop(diag=a >= qb)

        # diagonal tiles: fp16 probs, multiplicative causal mask, trimmed.
        # A full-width matmul opens the accumulation group (qb=0 only) and
        # another closes it.
        if qb == 0:
            diag_order = [(0, 0), (2, 256), (3, 384), (1, 0)]
        else:
            diag_order = [(1, 128), (2, 256), (3, 384), (0, 0)]
        for i, (r, off) in enumerate(diag_order):
            kt = 4 * qb + r
            st = do_st16(kt, off)
            fill.pop(diag=True)
            pt = pt_pool.tile([128, 1024], F16, tag="pt", name=f"pt{p}_{qb}_{kt}")
            if off:
                stv = st.rearrange("p (a n) -> p a n", a=2)
                ptv = pt.rearrange("p (a n) -> p a n", a=2)
                mkv = mask_sb[:, r, :].rearrange("p (a n) -> p a n", a=2)
                nc.scalar.activation(ptv[:, :, off:512], stv[:, :, off:512], EXP, scale=ES)
                nc.vector.tensor_mul(
                    ptv[:, :, off:512], ptv[:, :, off:512], mkv[:, :, off:512]
                )
            else:
                nc.scalar.activation(pt, st, EXP, scale=ES)
                nc.vector.tensor_mul(pt, pt, mask_sb[:, r, :])
            start = qb == 0 and i == 0
            stop = i == len(diag_order) - 1
            for hh in range(2):
                nc.tensor.matmul(
                    ots[hh][0 : D + 1, off:512],
                    lhsT=v_sb[:, kt, 2 * p + hh, :],
                    rhs=pt[:, hh * 512 + off : (hh + 1) * 512],
                    start=start,
                    stop=stop,
                    skip_group_check=not start,
                )
            fill.pop(diag=True)

        # normalization: copy both heads' OT out of PSUM, fp32 reciprocal of
        # the rowsum rows (which carry the 1/32 value-scale: the rowsum
        # column of V is 32.0), one DRAM-bounce broadcast, two gpsimd muls.
        # The last-emitted block's chain is the kernel tail: parallelize its
        # copies/muls across engines (ScalarE and DVE are idle by then).
        oc = sm_pool.tile([D + 1, 2, 512], F32, tag="oc", name=f"oc{p}_{qb}")
        nc.vector.tensor_copy(oc[:, 0, :], ots[0][0 : D + 1, :])
        if last:
            nc.scalar.copy(oc[:, 1, :], ots[1][0 : D + 1, :])
        else:
            nc.vector.tensor_copy(oc[:, 1, :], ots[1][0 : D + 1, :])
        if last:
            # tail-latency variant: bounce the RAW rowsums (skipping the
            # repartition DMA) and take the reciprocal after the broadcast
            # on the by-then-idle DVE - one less serial DMA in the final
            # normalize->Wo chain
            rd = dr_pool.tile([1, 2, 512], F32, tag="rd", name=f"rd{p}_{qb}")
            nc.sync.dma_start(out=rd, in_=oc[D : D + 1, :, :])
            rbr = sm_pool.tile([D, 2, 512], F32, tag="rbr", name=f"rbr{p}_{qb}",
                               bufs=1)
            nc.sync.dma_start(out=rbr, in_=rd.to_broadcast((D, 2, 512)))
            rbc = sm_pool.tile([D, 2, 512], F32, tag="rbc", name=f"rbc{p}_{qb}")
            nc.vector.reciprocal(rbc, rbr)
        else:
            rsq = sm_pool.tile([D, 16], F32, tag="rsq", name=f"rsq{p}_{qb}")
            nc.sync.dma_start(out=rsq, in_=oc[D : D + 1, :, :])
            rr = sm_pool.tile([D, 16], F32, tag="rr", name=f"rr{p}_{qb}")
            nc.vector.reciprocal(rr, rsq)
            rd = dr_pool.tile([1, 2, 512], F32, tag="rd", name=f"rd{p}_{qb}")
            nc.sync.dma_start(out=rd, in_=rr)
            rbc = sm_pool.tile([D, 2, 512], F32, tag="rbc", name=f"rbc{p}_{qb}")
            nc.sync.dma_start(out=rbc, in_=rd.to_broadcast((D, 2, 512)))
        stg = sm_pool.tile([D, 512], F16, tag="stg", name=f"stg{p}_{qb}")
        if last:
            nc.vector.tensor_mul(attnT_sb[0:D, p, qs], oc[0:D, 0, :], rbc[:, 0, :])
        else:
            nc.gpsimd.tensor_mul(attnT_sb[0:D, p, qs], oc[0:D, 0, :], rbc[:, 0, :])
        nc.gpsimd.tensor_mul(stg, oc[0:D, 1, :], rbc[:, 1, :])
        nc.sync.dma_start(out=attnT_sb[D:128, p, qs], in_=stg)

    # --- output projection: out tile = attnT.T @ wo ---
    def emit_wo_part(qt, tail=False):
        o_sb = ob_pool.tile([128, 1024], F16, tag="ob", name=f"ob{qt}")
        for nt in range(2):
            # in the tail, attention is done: borrow the idle score-PSUM pool
            # for every other tile to double the accumulate/copy rotation
            pool = st_ps if tail and nt == 1 else mm_ps
            tag = "st" if pool is st_ps else "mm"
            ps = pool.tile([128, 512], F32, tag=tag, name=f"ops{qt}_{nt}")
            for p in range(2):
                nc.tensor.matmul(
                    ps,
                    lhsT=attnT_sb[:, p, qt * 128 : (qt + 1) * 128],
                    rhs=wo_sb[:, p, nt * 512 : (nt + 1) * 512],
                    start=(p == 0),
                    stop=(p == 1),
                )
            # tail tiles split copies DVE/ScalarE (both idle at the end)
            if tail and nt == 1:
                nc.scalar.copy(o_sb[:, nt * 512 : (nt + 1) * 512], ps)
            else:
                nc.vector.tensor_copy(o_sb[:, nt * 512 : (nt + 1) * 512], ps)
        nc.sync.dma_start(out=out[qt * 128 : (qt + 1) * 128, :], in_=o_sb)

    class Fill:
        """FIFO of independent emission units; deps are enforced by the tile
        framework's semaphores, so order only affects performance. "late"
        units (Wo tiles, whose attnT producer chain is freshly emitted) pop
        only once mature - in the diagonal phase one block later, or anywhere
        two blocks later - so they neither head-of-line block the PE queue
        nor burst back-to-back (which would starve ScalarE via the mm-pool/
        DVE copy rotation)."""

        def __init__(self):
            self.q = []
            self.done = set()
            self.block = 0
            self.late_budget = 2

        def tick(self):
            self.block += 1
            self.late_budget = 2

        def add(self, key, fn, late=False):
            if key not in self.done:
                self.q.append((key, fn, self.block if late else -2))

        def pop(self, n=1, diag=False):
            for _ in range(n):
                idx = None
                fresh = False
                for i, (key, fn, birth) in enumerate(self.q):
                    if key in self.done:
                        continue
                    age = self.block - birth
                    if age < 1:
                        continue
                    if age == 1 and not (diag and self.late_budget > 0):
                        continue
                    idx = i
                    fresh = age == 1
                    break
                if idx is None:
                    return
                if fresh:
                    self.late_budget -= 1
                key, fn, _ = self.q.pop(idx)
                self.done.add(key)
                fn()

        def ensure(self, key, fn, late=False):
            if key not in self.done:
                self.done.add(key)
                fn()

        def flush(self):
            for key, fn, _ in list(self.q):
                if key not in self.done:
                    self.done.add(key)
                    fn()
            self.q = []

    fill = Fill()

    def qk_unit(p, qb, which):
        return (("qk", p, qb, which), lambda: emit_qk_part(p, qb, which))

    def v_unit(rt):
        return (("v", rt), lambda: emit_v(rt))

    tail_mode = [False]

    def wo_unit(qt):
        return (("wo", qt), lambda: emit_wo_part(qt, tail=tail_mode[0]))

    if phases is not None:
        # bisection mode: simple phase ordering, no fillers
        emit_presets()
        nofill = Fill()
        if phases is not None and "qk" in phases or phases is None:
            pass
        if "qk" in phases:
            for qb in range(NQB):
                for w in "qk":
                    emit_qk_part(0, qb, w)
                for w in "qk":
                    emit_qk_part(1, qb, w)
        if "v" in phases:
            for rt in range(NKT):
                emit_v(rt)
        if "attn" in phases:
            for qb in range(NQB):
                emit_attn(0, qb, nofill)
            for qb in range(NQB):
                emit_attn(1, qb, nofill)
        if "wo" in phases:
            for qt in range(NKT):
                emit_wo_part(qt)
    else:
        # pair-0 phase: V tiles, later pair-0 QK parts and early pair-1 QK
        # parts ride as fillers inside the exp-paced attention blocks
        for qb in range(NQB):
            fill.ensure(*qk_unit(0, qb, "q"))
            fill.ensure(*qk_unit(0, qb, "k"))
            for rt in range(4 * qb, 4 * qb + 4):
                fill.ensure(*v_unit(rt))
            if qb < NQB - 1:
                for rt in range(4 * qb + 4, 4 * qb + 8):
                    fill.add(*v_unit(rt))
                fill.add(*qk_unit(0, qb + 1, "q"))
                fill.add(*qk_unit(0, qb + 1, "k"))
            if qb == NQB - 1:
                # qk(1,0) must precede ALL pair-1 blocks: its k8 columns
                # 0:512 feed every pair-1 lower tile
                fill.add(*qk_unit(1, 0, "k"))
                fill.add(*qk_unit(1, 1, "q"))
                fill.add(*qk_unit(1, 1, "k"))
            emit_attn(0, qb, fill)
            if qb == 0:
                emit_presets()
        # pair-1 phase ordered (1, 2, 0, 3): each block's fillers are later
        # blocks' QK parts plus the Wo tiles unlocked so far ("late": popped
        # only once their attnT normalization has had time to land, rate-
        # capped to avoid bursts that starve ScalarE). Ending on qb=3 (the
        # largest block) absorbs the most Wo work; only its own norm chain
        # and Wo tiles (12-15) remain as the tail. Wo units are emitted
        # strictly after their producing normalization (emission order
        # defines dependency direction in the tile framework).
        fill.flush()
        fill.ensure(*qk_unit(1, 1, "q"))
        fill.ensure(*qk_unit(1, 1, "k"))
        fill.add(*qk_unit(1, 2, "q"))
        fill.add(*qk_unit(1, 2, "k"))
        emit_attn(1, 1, fill)
        fill.ensure(*qk_unit(1, 2, "q"))
        fill.ensure(*qk_unit(1, 2, "k"))
        fill.add(*qk_unit(1, 0, "q"))
        fill.add(*qk_unit(1, 0, "k"))
        fill.add(*qk_unit(1, 3, "q"))
        fill.add(*qk_unit(1, 3, "k"))
        for qt in range(4, 8):
            fill.add(*wo_unit(qt), late=True)
        emit_attn(1, 2, fill)
        fill.ensure(*qk_unit(1, 0, "q"))
        fill.ensure(*qk_unit(1, 0, "k"))
        for qt in range(8, 12):
            fill.add(*wo_unit(qt), late=True)
        emit_attn(1, 0, fill)
        fill.ensure(*qk_unit(1, 3, "q"))
        fill.ensure(*qk_unit(1, 3, "k"))
        for qt in range(0, 4):
            fill.add(*wo_unit(qt), late=True)
        emit_attn(1, 3, fill, last=True)
        tail_mode[0] = True
        fill.flush()
        for qt in range(12, 16):
            emit_wo_part(qt, tail=True)
    ctx.close()


def make_masks():
    i = np.arange(128)[:, None]
    j = np.arange(512)[None, :]
    m = np.stack([(i + 128 * r <= j) for r in range(4)], axis=0).astype(np.float16)
    return np.concatenate([m, m], axis=2)  # duplicated per head pair


def _hilo(a):
    hi = a.astype(NP8)
    lo = (a - hi.astype(np.float32)).astype(NP8)
    return hi, lo


def make_in_maps(x, Wq, bq, Wk, bk, Wv, bv, Wo):
    masks = make_masks()
    in_maps = []
    x8b = []
    for b in range(2):
        xT = np.ascontiguousarray(x[b].T.astype(np.float32))
        xh, xl = _hilo(xT)
        x8b.append(np.ascontiguousarray(np.stack([xh, xl], axis=1)))
    wsplits = {}
    for nm, W in (("q", Wq), ("k", Wk), ("v", Wv)):
        for hg in range(4):
            sl = slice(hg * S, (hg + 1) * S)
            wh, wl = _hilo(np.asarray(W[:, sl], np.float32) * SC)
            whd = np.ascontiguousarray(np.stack([wh, wh], axis=1))
            wsplits[(nm, hg)] = (whd, np.ascontiguousarray(wl))
    for c in range(8):
        b, hg = divmod(c, 4)
        sl = slice(hg * S, (hg + 1) * S)
        wqh_, wql_ = wsplits[("q", hg)]
        wkh_, wkl_ = wsplits[("k", hg)]
        wvh_, wvl_ = wsplits[("v", hg)]
        in_maps.append(
            {
                "x8": x8b[b],
                "wqh": wqh_, "wql": wql_,
                "wkh": wkh_, "wkl": wkl_,
                "wvh": wvh_, "wvl": wvl_,
                "wo": np.ascontiguousarray(Wo[sl, :].astype(np.float16)),
                "bq": np.ascontiguousarray((SC * bq[sl]).astype(np.float32).reshape(S, 1)),
                "bk": np.ascontiguousarray((SC * bk[sl]).astype(np.float32).reshape(S, 1)),
                "bv": np.ascontiguousarray((SC * bv[sl]).astype(np.float32).reshape(1, S)),
                "masks": masks,
            }
        )
    return in_maps


_NC_CACHE = None


def _get_nc():
    global _NC_CACHE
    if _NC_CACHE is None:
        _NC_CACHE = build_nc()
    return _NC_CACHE


def _run(x, Wq, bq, Wk, bk, Wv, bv, Wo, bo, trace=False, **spmd_kwargs):
    nc = _get_nc()
    in_maps = make_in_maps(
        np.asarray(x), np.asarray(Wq), np.asarray(bq), np.asarray(Wk),
        np.asarray(bk), np.asarray(Wv), np.asarray(bv), np.asarray(Wo),
    )
    res = run_bass_kernel_spmd(
        nc, in_maps, core_ids=list(range(8)), trace=trace, **spmd_kwargs
    )
    out = np.zeros((2, T, E), dtype=np.float32)
    for c in range(8):
        out[c // 4] += res.results[c]["out"]
    out += np.asarray(bo, dtype=np.float32)[None, None, :]
    return out, res


def kernel(x, Wq, bq, Wk, bk, Wv, bv, Wo, bo):
    out, _ = _run(x, Wq, bq, Wk, bk, Wv, bv, Wo, bo)
    return out


# revision 58
# speedup vs baseline: 1.1224x; 1.0039x over previous
"""Causal self-attention (B=2, T=2048, E=1024, H=16, D=64) on 8 TRN2 NeuronCores.

Sharding: core = (batch b, head-group hg): 2 batches x 4 head-groups of 4 heads.
Each core computes QKV projections for its 4 heads (256 columns), causal
attention, and the output projection against its 256 rows of Wo, producing a
partial [2048, 1024] output. Host sums the 4 head-group partials per batch
(the tensor-parallel all-reduce) and adds bo.

Precision/performance structure (matmul cost goes by output columns; fp8e4
DoubleRow pairs two K-tiles per instruction at half cost):
  - QKV projections in fp8 hi+lo split precision: W' = 32*W and x decompose
    as hi8 + lo8 host-side; (whi+wlo)@x_hi + whi@x_lo runs as DoubleRow pairs
    for 0.75x the fp16 cost at ~fp16 accuracy. The 32x weight scale keeps lo
    residuals above e4m3's subnormal floor; it is folded back via the exp
    scale and V's 32.0 rowsum column.
  - Scores per head pair: diagonal-block tiles fp16; strictly-lower tiles use
    fp8 q/k via a zero-padded DoubleRow (half cost). Lower-tile probs
    quantize to fp8 and feed DoubleRow AV against V's hi8+lo8 split (half
    cost); diagonal tiles stay fp16 (multiplicative causal mask), so every
    query's nearest <=512 keys are full precision - far-key quantization
    noise averages out in the softmax.
  - Emission interleaves independent matmul "filler" units (V tiles, next
    QK parts, output-projection tiles) into the attention instruction stream:
    the PE queue is in-order, so without fillers it head-of-line blocks on
    ScalarE exp between score and AV instructions.
"""
from contextlib import ExitStack

import numpy as np
import ml_dtypes

import concourse.bass as bass  # noqa: F401
import concourse.mybir as mybir
import concourse.tile as tile
from concourse import bacc
from concourse.bass_utils import run_bass_kernel_spmd

T = 2048
E = 1024
HPC = 4          # heads per core
D = 64
S = HPC * D      # 256: per-core head-column slice
KE = E // 128    # 8 contraction tiles for the projections
NKT = T // 128   # 16 key row tiles
NQB = T // 512   # 4 query column blocks
VP = 96          # padded V columns for DoubleRow AV (64 vals + rowsum + pad)
SC = 32.0        # weight scale: keeps fp8 lo residuals out of subnormals
ES = 0.125 / (SC * SC)  # exp scale with the q/k scale folded in
F8 = mybir.dt.float8e4
F16 = mybir.dt.float16
F32 = mybir.dt.float32
EXP = mybir.ActivationFunctionType.Exp
DR = mybir.MatmulPerfMode.DoubleRow
NP8 = ml_dtypes.float8_e4m3fn


def build_nc(phases=None):
    nc = bacc.Bacc("TRN2", target_bir_lowering=False, debug=False)
    x8 = nc.dram_tensor("x8", [E, 2, T], F8, kind="ExternalInput").ap()
    wqh = nc.dram_tensor("wqh", [E, 2, S], F8, kind="ExternalInput").ap()
    wkh = nc.dram_tensor("wkh", [E, 2, S], F8, kind="ExternalInput").ap()
    wvh = nc.dram_tensor("wvh", [E, 2, S], F8, kind="ExternalInput").ap()
    wql = nc.dram_tensor("wql", [E, S], F8, kind="ExternalInput").ap()
    wkl = nc.dram_tensor("wkl", [E, S], F8, kind="ExternalInput").ap()
    wvl = nc.dram_tensor("wvl", [E, S], F8, kind="ExternalInput").ap()
    wo = nc.dram_tensor("wo", [S, E], F16, kind="ExternalInput").ap()
    bq = nc.dram_tensor("bq", [S, 1], F32, kind="ExternalInput").ap()
    bk = nc.dram_tensor("bk", [S, 1], F32, kind="ExternalInput").ap()
    bv = nc.dram_tensor("bv", [1, S], F32, kind="ExternalInput").ap()
    masks = nc.dram_tensor("masks", [4, 128, 1024], F16, kind="ExternalInput").ap()
    out = nc.dram_tensor("out", [T, E], F16, kind="ExternalOutput").ap()

    with tile.TileContext(nc) as tc:
        _emit(nc, tc, x8, (wqh, wkh, wvh), (wql, wkl, wvl), wo, bq, bk, bv,
              masks, out, phases=phases)
    nc.compile()
    return nc


def _emit(nc, tc, x8, whs, wls, wo, bq, bk, bv, masks, out, phases=None):
    wqh, wkh, wvh = whs
    wql, wkl, wvl = wls
    ctx = ExitStack()
    consts = ctx.enter_context(tc.tile_pool(name="consts", bufs=1))
    mm_ps = ctx.enter_context(tc.tile_pool(name="mm_ps", bufs=2, space="PSUM"))
    st_ps = ctx.enter_context(tc.tile_pool(name="st_ps", bufs=2, space="PSUM"))
    ot_ps = ctx.enter_context(tc.tile_pool(name="ot_ps", bufs=2, space="PSUM"))
    pt_pool = ctx.enter_context(tc.tile_pool(name="pt", bufs=6))
    p8_pool = ctx.enter_context(tc.tile_pool(name="p8", bufs=6))
    sm_pool = ctx.enter_context(tc.tile_pool(name="sm", bufs=4))
    ob_pool = ctx.enter_context(tc.tile_pool(name="ob", bufs=4))
    dr_pool = ctx.enter_context(tc.tile_pool(name="dr", bufs=8, space="DRAM"))

    # --- constant tiles ---
    x_sb = consts.tile([128, KE, 2, T], F8)       # (hi, lo) of x^T
    wqh_sb = consts.tile([128, KE, 2, S], F8)     # duplicated hi weights
    wkh_sb = consts.tile([128, KE, 2, S], F8)
    wvh_sb = consts.tile([128, KE, 2, S], F8)
    wql_sb = consts.tile([128, KE, S], F8)        # lo weights
    wkl_sb = consts.tile([128, KE, S], F8)
    wvl_sb = consts.tile([128, KE, S], F8)
    wo_sb = consts.tile([128, S // 128, E], F16)
    bq_sb = consts.tile([128, 2], F32)
    bk_sb = consts.tile([128, 2], F32)
    bv_bc = consts.tile([128, S], F32)
    mask_sb = consts.tile([128, 4, 1024], F16)
    qt_sb = consts.tile([128, 2, T], F16)
    kt_sb = consts.tile([128, 2, T], F16)
    q8_sb = consts.tile([128, 2, 2, T], F8)       # fp8 q, slot 1 zero-padded
    k8_sb = consts.tile([128, 2, 2, T], F8)       # fp8 k, slot 1 zero-padded
    v_sb = consts.tile([128, NKT, HPC, D + 1], F16)
    v8h_sb = consts.tile([128, NKT, HPC, VP], F8)
    v8l_sb = consts.tile([128, NKT, HPC, VP], F8)
    attnT_sb = consts.tile([128, 2, T], F16)

    # batched constant loads, ordered so the first QK group can start after
    # wq + the first x query-block instead of the full x tensor
    xr = x8.rearrange("(ke p) two t -> p ke two t", p=128)
    wqhr = wqh.rearrange("(ke p) two s -> p ke two s", p=128)
    nc.sync.dma_start(out=wqh_sb[:, 0:4], in_=wqhr[:, 0:4])
    for i in range(2):
        nc.sync.dma_start(out=x_sb[:, 0:4, i, 0:512], in_=xr[:, 0:4, i, 0:512])
    nc.sync.dma_start(out=wqh_sb[:, 4:8], in_=wqhr[:, 4:8])
    for i in range(2):
        nc.sync.dma_start(out=x_sb[:, 4:8, i, 0:512], in_=xr[:, 4:8, i, 0:512])
    # order: everything attn(0,0) needs (q, k, masks, v) before the
    # remaining x query blocks; the start is DMA-serial so order = latency
    nc.sync.dma_start(out=wql_sb, in_=wql.rearrange("(ke p) s -> p ke s", p=128))
    nc.sync.dma_start(out=bq_sb, in_=bq.rearrange("(a p) one -> p (a one)", p=128))
    nc.sync.dma_start(out=wkh_sb, in_=wkh.rearrange("(ke p) two s -> p ke two s", p=128))
    nc.sync.dma_start(out=wkl_sb, in_=wkl.rearrange("(ke p) s -> p ke s", p=128))
    nc.sync.dma_start(out=bk_sb, in_=bk.rearrange("(a p) one -> p (a one)", p=128))
    # masks split: only r=0 gates attn(0,0)'s first masked tile; r1-3 follow
    # the V weights so the first AV chain isn't serialized behind the full 1MB
    mr = masks.rearrange("r p n -> p r n")
    nc.sync.dma_start(out=mask_sb[:, 0:1], in_=mr[:, 0:1])
    nc.sync.dma_start(out=wvh_sb, in_=wvh.rearrange("(ke p) two s -> p ke two s", p=128))
    nc.sync.dma_start(out=wvl_sb, in_=wvl.rearrange("(ke p) s -> p ke s", p=128))
    nc.sync.dma_start(out=bv_bc, in_=bv.to_broadcast((128, S)))
    nc.sync.dma_start(out=mask_sb[:, 1:4], in_=mr[:, 1:4])
    for qb in range(1, NQB):
        qs = slice(qb * 512, (qb + 1) * 512)
        for i in range(2):
            nc.sync.dma_start(out=x_sb[:, :, i, qs], in_=xr[:, :, i, qs])
    nc.sync.dma_start(out=wo_sb, in_=wo.rearrange("(a p) n -> p a n", p=128))
    nc.vector.memset(v_sb[:, :, :, D : D + 1], SC)
    # warm the exp activation table during the initial DMA wait
    warm = consts.tile([1, 1], F32)
    nc.vector.memset(warm, 0.0)
    nc.scalar.activation(warm, warm, EXP)

    def emit_presets():
        # zero pads/slots read by the DoubleRow matmuls; not needed until the
        # first lower-tile block, so these sit after attn(0,0) in Pool's
        # queue (they are disjoint from the slot-0/value writes)
        nc.gpsimd.memset(v8h_sb[:, :, :, D : D + 1], SC)
        nc.gpsimd.memset(v8h_sb[:, :, :, D + 1 : VP], 0.0)
        nc.gpsimd.memset(v8l_sb[:, :, :, D : VP], 0.0)
        nc.gpsimd.memset(q8_sb[:, :, 1, :], 0.0)
        nc.gpsimd.memset(k8_sb[:, :, 1, :], 0.0)

    # --- V = x @ wv + bv (fp8 hi-lo DoubleRow), plus fp8 hi/lo splits ---
    def emit_v(rt):
        rsl = slice(rt * 128, (rt + 1) * 128)
        ps = mm_ps.tile([128, 512], F32, tag="mm", name=f"vps{rt}")
        for ke in range(KE):
            nc.tensor.matmul(
                ps[:, 0:S],
                lhsT=x_sb[:, ke, :, rsl],
                rhs=wvh_sb[:, ke],
                start=(ke == 0),
                stop=False,
                perf_mode=DR,
            )
        for a in range(KE // 2):
            nc.tensor.matmul(
                ps[:, 0:S],
                lhsT=x_sb[:, 2 * a : 2 * a + 2, 0, rsl],
                rhs=wvl_sb[:, 2 * a : 2 * a + 2, :],
                start=False,
                stop=(a == KE // 2 - 1),
                perf_mode=DR,
            )
        nc.vector.tensor_add(
            v_sb[:, rt, :, 0:D],
            ps[:, 0:S].rearrange("p (h d) -> p h d", h=HPC),
            bv_bc.rearrange("p (h d) -> p h d", h=HPC),
        )
        nc.gpsimd.tensor_copy(v8h_sb[:, rt, :, 0:D], v_sb[:, rt, :, 0:D])
        nc.gpsimd.tensor_sub(
            v8l_sb[:, rt, :, 0:D], v_sb[:, rt, :, 0:D], v8h_sb[:, rt, :, 0:D]
        )

    # --- QT/KT = (x @ w + b).T for one pair of heads (128 cols) ---
    def emit_qk_part(p, qb, which):
        qs = slice(qb * 512, (qb + 1) * 512)
        psl = slice(p * 128, (p + 1) * 128)
        wh_sb, wl_sb, b_sb, dst, nm = {
            "q": (wqh_sb, wql_sb, bq_sb, qt_sb, "q"),
            "k": (wkh_sb, wkl_sb, bk_sb, kt_sb, "k"),
        }[which]
        ps = mm_ps.tile([128, 512], F32, tag="mm", name=f"{nm}ps{p}_{qb}")
        for ke in range(KE):
            nc.tensor.matmul(
                ps,
                lhsT=wh_sb[:, ke, :, psl],
                rhs=x_sb[:, ke, :, qs],
                start=(ke == 0),
                stop=False,
                perf_mode=DR,
            )
        for a in range(KE // 2):
            nc.tensor.matmul(
                ps,
                lhsT=wl_sb[:, 2 * a : 2 * a + 2, psl],
                rhs=x_sb[:, 2 * a : 2 * a + 2, 0, qs],
                start=False,
                stop=(a == KE // 2 - 1),
                perf_mode=DR,
            )
        nc.vector.tensor_scalar_add(dst[:, p, qs], ps, b_sb[:, p : p + 1])
        # fp8 copies for the zero-padded DoubleRow lower-tile scores:
        # q8 needed for query blocks >=1, k8 for key blocks <=2
        if which == "q" and qb > 0:
            nc.vector.tensor_copy(q8_sb[:, p, 0, qs], dst[:, p, qs])
        if which == "k" and qb < 3:
            nc.gpsimd.tensor_copy(k8_sb[:, p, 0, qs], dst[:, p, qs])

    # --- attention for pair p (heads 2p, 2p+1), query block qb ---
    def emit_attn(p, qb, fill, last=False):
        fill.tick()
        qs = slice(qb * 512, (qb + 1) * 512)
        ots = [
            ot_ps.tile([VP, 512], F32, tag="ot", name=f"ot{p}_{qb}_{i}")
            for i in range(2)
        ]

        def do_st8(kt):
            # fp8 zero-padded DoubleRow score tile (strictly-lower keys)
            st = st_ps.tile([128, 1024], F32, tag="st", name=f"st{p}_{qb}_{kt}")
            for hh in range(2):
                hs = slice(hh * 64, (hh + 1) * 64)
                nc.tensor.matmul(
                    st[:, hh * 512 : (hh + 1) * 512],
                    lhsT=k8_sb[hs, p, :, kt * 128 : (kt + 1) * 128],
                    rhs=q8_sb[hs, p, :, qs],
                    start=True,
                    stop=True,
                    perf_mode=DR,
                )
            return st

        def do_st16(kt, off):
            st = st_ps.tile([128, 1024], F32, tag="st", name=f"st{p}_{qb}_{kt}")
            for hh in range(2):
                hs = slice(hh * 64, (hh + 1) * 64)
                nc.tensor.matmul(
                    st[:, hh * 512 + off : (hh + 1) * 512],
                    lhsT=kt_sb[hs, p, kt * 128 : (kt + 1) * 128],
                    rhs=qt_sb[hs, p, qb * 512 + off : (qb + 1) * 512],
                    start=True,
                    stop=True,
                )
            return st

        # lower key tiles: fp8 probs, DoubleRow AV over (hi, lo) V splits
        for a in range(2 * qb):
            st0 = do_st8(2 * a)
            fill.pop(diag=a >= qb)
            st1 = do_st8(2 * a + 1)
            pt8 = p8_pool.tile([128, 2, 1024], F8, tag="p8", name=f"p8_{p}_{qb}_{a}")
            nc.scalar.activation(pt8[:, 0, :], st0, EXP, scale=ES)
            nc.scalar.activation(pt8[:, 1, :], st1, EXP, scale=ES)
            for hh in range(2):
                rhs8 = pt8[:, :, hh * 512 : (hh + 1) * 512]
                for v8 in (v8h_sb, v8l_sb):
                    nc.tensor.matmul(
                        ots[hh],
                        lhsT=v8[:, 2 * a : 2 * a + 2, 2 * p + hh, :],
                        rhs=rhs8,
                        start=(a == 0 and v8 is v8h_sb),
                        stop=False,
                        perf_mode=DR,
                        skip_group_check=(a != 0 or v8 is not v8h_sb),
                    )
            fill.pop(diag=a >= qb)

        # diagonal tiles: fp16 probs, multiplicative causal mask, trimmed.
        # A full-width matmul opens the accumulation group (qb=0 only) and
        # another closes it.
        if qb == 0:
            diag_order = [(0, 0), (2, 256), (3, 384), (1, 0)]
        else:
            diag_order = [(1, 128), (2, 256), (3, 384), (0, 0)]
        for i, (r, off) in enumerate(diag_order):
            kt = 4 * qb + r
            st = do_st16(kt, off)
            fill.pop(diag=True)
            pt = pt_pool.tile([128, 1024], F16, tag="pt", name=f"pt{p}_{qb}_{kt}")
            if off:
                stv = st.rearrange("p (a n) -> p a n", a=2)
                ptv = pt.rearrange("p (a n) -> p a n", a=2)
                mkv = mask_sb[:, r, :].rearrange("p (a n) -> p a n", a=2)
                nc.scalar.activation(ptv[:, :, off:512], stv[:, :, off:512], EXP, scale=ES)
                nc.vector.tensor_mul(
                    ptv[:, :, off:512], ptv[:, :, off:512], mkv[:, :, off:512]
                )
            else:
                nc.scalar.activation(pt, st, EXP, scale=ES)
                nc.vector.tensor_mul(pt, pt, mask_sb[:, r, :])
            start = qb == 0 and i == 0
            stop = i == len(diag_order) - 1
            for hh in range(2):
                nc.tensor.matmul(
                    ots[hh][0 : D + 1, off:512],
                    lhsT=v_sb[:, kt, 2 * p + hh, :],
                    rhs=pt[:, hh * 512 + off : (hh + 1) * 512],
                    start=start,
                    stop=stop,
                    skip_group_check=not start,
                )
            fill.pop(diag=True)

        # normalization: copy both heads' OT out of PSUM, fp32 reciprocal of
        # the rowsum rows (which carry the 1/32 value-scale: the rowsum
        # column of V is 32.0), one DRAM-bounce broadcast, two gpsimd muls.
        # The last-emitted block's chain is the kernel tail: parallelize its
        # copies/muls across engines (ScalarE and DVE are idle by then).
        oc = sm_pool.tile([D + 1, 2, 512], F32, tag="oc", name=f"oc{p}_{qb}")
        nc.vector.tensor_copy(oc[:, 0, :], ots[0][0 : D + 1, :])
        if last:
            nc.scalar.copy(oc[:, 1, :], ots[1][0 : D + 1, :])
        else:
            nc.vector.tensor_copy(oc[:, 1, :], ots[1][0 : D + 1, :])
        if last:
            # tail-latency variant: bounce the RAW rowsums (skipping the
            # repartition DMA) and take the reciprocal after the broadcast
            # on the by-then-idle DVE - one less serial DMA in the final
            # normalize->Wo chain
            rd = dr_pool.tile([1, 2, 512], F32, tag="rd", name=f"rd{p}_{qb}")
            nc.sync.dma_start(out=rd, in_=oc[D : D + 1, :, :])
            rbr = sm_pool.tile([D, 2, 512], F32, tag="rbr", name=f"rbr{p}_{qb}",
                               bufs=1)
            nc.sync.dma_start(out=rbr, in_=rd.to_broadcast((D, 2, 512)))
            rbc = sm_pool.tile([D, 2, 512], F32, tag="rbc", name=f"rbc{p}_{qb}")
            # head 1 first: it feeds the serial stg-DMA chain the tail Wo
            # tiles wait on; head 0's reciprocal runs during the Pool mul
            nc.vector.reciprocal(rbc[:, 1, :], rbr[:, 1, :])
            nc.vector.reciprocal(rbc[:, 0, :], rbr[:, 0, :])
        else:
            rsq = sm_pool.tile([D, 16], F32, tag="rsq", name=f"rsq{p}_{qb}")
            nc.sync.dma_start(out=rsq, in_=oc[D : D + 1, :, :])
            rr = sm_pool.tile([D, 16], F32, tag="rr", name=f"rr{p}_{qb}")
            nc.vector.reciprocal(rr, rsq)
            rd = dr_pool.tile([1, 2, 512], F32, tag="rd", name=f"rd{p}_{qb}")
            nc.sync.dma_start(out=rd, in_=rr)
            rbc = sm_pool.tile([D, 2, 512], F32, tag="rbc", name=f"rbc{p}_{qb}")
            nc.sync.dma_start(out=rbc, in_=rd.to_broadcast((D, 2, 512)))
        stg = sm_pool.tile([D, 512], F16, tag="stg", name=f"stg{p}_{qb}")
        if last:
            # emit the stg chain first: it is the tail's critical path
            nc.gpsimd.tensor_mul(stg, oc[0:D, 1, :], rbc[:, 1, :])
            nc.sync.dma_start(out=attnT_sb[D:128, p, qs], in_=stg)
            nc.vector.tensor_mul(attnT_sb[0:D, p, qs], oc[0:D, 0, :], rbc[:, 0, :])
        else:
            nc.gpsimd.tensor_mul(attnT_sb[0:D, p, qs], oc[0:D, 0, :], rbc[:, 0, :])
            nc.gpsimd.tensor_mul(stg, oc[0:D, 1, :], rbc[:, 1, :])
            nc.sync.dma_start(out=attnT_sb[D:128, p, qs], in_=stg)

    # --- output projection: out tile = attnT.T @ wo ---
    def emit_wo_part(qt, tail=False):
        o_sb = ob_pool.tile([128, 1024], F16, tag="ob", name=f"ob{qt}")
        for nt in range(2):
            # in the tail, attention is done: borrow the idle score-PSUM pool
            # for every other tile to double the accumulate/copy rotation
            pool = st_ps if tail and nt == 1 else mm_ps
            tag = "st" if pool is st_ps else "mm"
            ps = pool.tile([128, 512], F32, tag=tag, name=f"ops{qt}_{nt}")
            for p in range(2):
                nc.tensor.matmul(
                    ps,
                    lhsT=attnT_sb[:, p, qt * 128 : (qt + 1) * 128],
                    rhs=wo_sb[:, p, nt * 512 : (nt + 1) * 512],
                    start=(p == 0),
                    stop=(p == 1),
                )
            # tail tiles split copies DVE/ScalarE (both idle at the end)
            if tail and nt == 1:
                nc.scalar.copy(o_sb[:, nt * 512 : (nt + 1) * 512], ps)
            else:
                nc.vector.tensor_copy(o_sb[:, nt * 512 : (nt + 1) * 512], ps)
        nc.sync.dma_start(out=out[qt * 128 : (qt + 1) * 128, :], in_=o_sb)

    class Fill:
        """FIFO of independent emission units; deps are enforced by the tile
        framework's semaphores, so order only affects performance. "late"
        units (Wo tiles, whose attnT producer chain is freshly emitted) pop
        only once mature - in the diagonal phase one block later, or anywhere
        two blocks later - so they neither head-of-line block the PE queue
        nor burst back-to-back (which would starve ScalarE via the mm-pool/
        DVE copy rotation)."""

        def __init__(self):
            self.q = []
            self.done = set()
            self.block = 0
            self.late_budget = 2

        def tick(self):
            self.block += 1
            self.late_budget = 2

        def add(self, key, fn, late=False):
            if key not in self.done:
                self.q.append((key, fn, self.block if late else -2))

        def pop(self, n=1, diag=False):
            for _ in range(n):
                idx = None
                fresh = False
                for i, (key, fn, birth) in enumerate(self.q):
                    if key in self.done:
                        continue
                    age = self.block - birth
                    if age < 1:
                        continue
                    if age == 1 and not (diag and self.late_budget > 0):
                        continue
                    idx = i
                    fresh = age == 1
                    break
                if idx is None:
                    return
                if fresh:
                    self.late_budget -= 1
                key, fn, _ = self.q.pop(idx)
                self.done.add(key)
                fn()

        def ensure(self, key, fn, late=False):
            if key not in self.done:
                self.done.add(key)
                fn()

        def flush(self):
            for key, fn, _ in list(self.q):
                if key not in self.done:
                    self.done.add(key)
                    fn()
            self.q = []

    fill = Fill()

    def qk_unit(p, qb, which):
        return (("qk", p, qb, which), lambda: emit_qk_part(p, qb, which))

    def v_unit(rt):
        return (("v", rt), lambda: emit_v(rt))

    tail_mode = [False]

    def wo_unit(qt):
        return (("wo", qt), lambda: emit_wo_part(qt, tail=tail_mode[0]))

    if phases is not None:
        # bisection mode: simple phase ordering, no fillers
        emit_presets()
        nofill = Fill()
        if phases is not None and "qk" in phases or phases is None:
            pass
        if "qk" in phases:
            for qb in range(NQB):
                for w in "qk":
                    emit_qk_part(0, qb, w)
                for w in "qk":
                    emit_qk_part(1, qb, w)
        if "v" in phases:
            for rt in range(NKT):
                emit_v(rt)
        if "attn" in phases:
            for qb in range(NQB):
                emit_attn(0, qb, nofill)
            for qb in range(NQB):
                emit_attn(1, qb, nofill)
        if "wo" in phases:
            for qt in range(NKT):
                emit_wo_part(qt)
    else:
        # pair-0 phase: V tiles, later pair-0 QK parts and early pair-1 QK
        # parts ride as fillers inside the exp-paced attention blocks
        for qb in range(NQB):
            fill.ensure(*qk_unit(0, qb, "q"))
            fill.ensure(*qk_unit(0, qb, "k"))
            for rt in range(4 * qb, 4 * qb + 4):
                fill.ensure(*v_unit(rt))
            if qb < NQB - 1:
                for rt in range(4 * qb + 4, 4 * qb + 8):
                    fill.add(*v_unit(rt))
                fill.add(*qk_unit(0, qb + 1, "q"))
                fill.add(*qk_unit(0, qb + 1, "k"))
            if qb == NQB - 1:
                # qk(1,0) must precede ALL pair-1 blocks: its k8 columns
                # 0:512 feed every pair-1 lower tile
                fill.add(*qk_unit(1, 0, "k"))
                fill.add(*qk_unit(1, 1, "q"))
                fill.add(*qk_unit(1, 1, "k"))
            emit_attn(0, qb, fill)
            if qb == 0:
                emit_presets()
        # pair-1 phase ordered (1, 2, 0, 3): each block's fillers are later
        # blocks' QK parts plus the Wo tiles unlocked so far ("late": popped
        # only once their attnT normalization has had time to land, rate-
        # capped to avoid bursts that starve ScalarE). Ending on qb=3 (the
        # largest block) absorbs the most Wo work; only its own norm chain
        # and Wo tiles (12-15) remain as the tail. Wo units are emitted
        # strictly after their producing normalization (emission order
        # defines dependency direction in the tile framework).
        fill.flush()
        fill.ensure(*qk_unit(1, 1, "q"))
        fill.ensure(*qk_unit(1, 1, "k"))
        fill.add(*qk_unit(1, 2, "q"))
        fill.add(*qk_unit(1, 2, "k"))
        emit_attn(1, 1, fill)
        fill.ensure(*qk_unit(1, 2, "q"))
        fill.ensure(*qk_unit(1, 2, "k"))
        fill.add(*qk_unit(1, 0, "q"))
        fill.add(*qk_unit(1, 0, "k"))
        fill.add(*qk_unit(1, 3, "q"))
        fill.add(*qk_unit(1, 3, "k"))
        for qt in range(4, 8):
            fill.add(*wo_unit(qt), late=True)
        emit_attn(1, 2, fill)
        fill.ensure(*qk_unit(1, 0, "q"))
        fill.ensure(*qk_unit(1, 0, "k"))
        for qt in range(8, 12):
            fill.add(*wo_unit(qt), late=True)
        emit_attn(1, 0, fill)
        fill.ensure(*qk_unit(1, 3, "q"))
        fill.ensure(*qk_unit(1, 3, "k"))
        for qt in range(0, 4):
            fill.add(*wo_unit(qt), late=True)
        emit_attn(1, 3, fill, last=True)
        tail_mode[0] = True
        fill.flush()
        for qt in range(12, 16):
            emit_wo_part(qt, tail=True)
    ctx.close()


def make_masks():
    i = np.arange(128)[:, None]
    j = np.arange(512)[None, :]
    m = np.stack([(i + 128 * r <= j) for r in range(4)], axis=0).astype(np.float16)
    return np.concatenate([m, m], axis=2)  # duplicated per head pair


def _hilo(a):
    hi = a.astype(NP8)
    lo = (a - hi.astype(np.float32)).astype(NP8)
    return hi, lo


def make_in_maps(x, Wq, bq, Wk, bk, Wv, bv, Wo):
    masks = make_masks()
    in_maps = []
    x8b = []
    for b in range(2):
        xT = np.ascontiguousarray(x[b].T.astype(np.float32))
        xh, xl = _hilo(xT)
        x8b.append(np.ascontiguousarray(np.stack([xh, xl], axis=1)))
    wsplits = {}
    for nm, W in (("q", Wq), ("k", Wk), ("v", Wv)):
        for hg in range(4):
            sl = slice(hg * S, (hg + 1) * S)
            wh, wl = _hilo(np.asarray(W[:, sl], np.float32) * SC)
            whd = np.ascontiguousarray(np.stack([wh, wh], axis=1))
            wsplits[(nm, hg)] = (whd, np.ascontiguousarray(wl))
    for c in range(8):
        b, hg = divmod(c, 4)
        sl = slice(hg * S, (hg + 1) * S)
        wqh_, wql_ = wsplits[("q", hg)]
        wkh_, wkl_ = wsplits[("k", hg)]
        wvh_, wvl_ = wsplits[("v", hg)]
        in_maps.append(
            {
                "x8": x8b[b],
                "wqh": wqh_, "wql": wql_,
                "wkh": wkh_, "wkl": wkl_,
                "wvh": wvh_, "wvl": wvl_,
                "wo": np.ascontiguousarray(Wo[sl, :].astype(np.float16)),
                "bq": np.ascontiguousarray((SC * bq[sl]).astype(np.float32).reshape(S, 1)),
                "bk": np.ascontiguousarray((SC * bk[sl]).astype(np.float32).reshape(S, 1)),
                "bv": np.ascontiguousarray((SC * bv[sl]).astype(np.float32).reshape(1, S)),
                "masks": masks,
            }
        )
    return in_maps


_NC_CACHE = None


def _get_nc():
    global _NC_CACHE
    if _NC_CACHE is None:
        _NC_CACHE = build_nc()
    return _NC_CACHE


def _run(x, Wq, bq, Wk, bk, Wv, bv, Wo, bo, trace=False, **spmd_kwargs):
    nc = _get_nc()
    in_maps = make_in_maps(
        np.asarray(x), np.asarray(Wq), np.asarray(bq), np.asarray(Wk),
        np.asarray(bk), np.asarray(Wv), np.asarray(bv), np.asarray(Wo),
    )
    res = run_bass_kernel_spmd(
        nc, in_maps, core_ids=list(range(8)), trace=trace, **spmd_kwargs
    )
    out = np.zeros((2, T, E), dtype=np.float32)
    for c in range(8):
        out[c // 4] += res.results[c]["out"]
    out += np.asarray(bo, dtype=np.float32)[None, None, :]
    return out, res


def kernel(x, Wq, bq, Wk, bk, Wv, bv, Wo, bo):
    out, _ = _run(x, Wq, bq, Wk, bk, Wv, bv, Wo, bo)
    return out
